# revision 11
# baseline (speedup 1.0000x reference)
"""3-layer GraphSAGE + classifier + log_softmax on 8 Trainium2 NeuronCores.

Self-contained: host-side sharding/packing + Bass/Tile device kernel.

Strategy
--------
concat([x, agg]) @ W  ==  x @ W_top + Ahat @ (x @ W_bot)   (linearity)
so aggregation happens in the 256-dim projected space.

- Nodes are permuted into 704 tiles of 128 (in-degree balanced), 88 tiles/core.
- Per layer: phase A computes r = x@W_top + b and p = x@W_bot per owned tile
  (PE matmuls, fp32r for layer 1, fp16 after), p is written fp16 and
  AllGathered so every core holds the full p table in DRAM.
- Phase B: per dst tile, gather p[src] rows for its in-edges via dma_gather
  (int16 indices -> three overlapping 32768-row windows), build a one-hot
  selection matrix S[e, d] = wn_e * (dst_local_e == d) on DVE, and accumulate
  agg = sum_c S_c.T @ msg_c on the PE into PSUM.  x_next = relu(agg + r).
- x_next is transposed on the PE (2x 128x128) to feed the next layer's
  stationary operand; the classifier (768->7) + log_softmax are fused into
  layer-3 phase B.
"""

import numpy as np

import concourse.bass as bass
import concourse.mybir as mybir
import concourse.tile as tile
from concourse import bacc
from concourse.bass_utils import run_bass_kernel_spmd
from concourse.masks import make_identity

# problem constants
N = 89250
IN_F = 500
HID = 256
NCLS = 7
FPAD = 512  # padded input feature dim

NC = 8  # cores
P = 128
NT = 704  # node tiles
TPC = NT // NC  # 88 tiles per core
NPAD = NT * P  # 90112
NPC = TPC * P  # 11264 nodes per core
G = 2  # tiles per gather group
NGRP = TPC // G

WBASE = (0, 28672, 57344)  # gather window base rows
WCAP = 32768  # int16 index reach

f32 = mybir.dt.float32
f32r = mybir.dt.float32r
f16 = mybir.dt.float16
i16 = mybir.dt.int16
i32 = mybir.dt.int32

_compile_cache = {}


# --------------------------------------------------------------------------
# host-side prep
# --------------------------------------------------------------------------

def _assign_tiles(in_deg):
    """LPT: assign node ids (0..NPAD) to (tile, slot), balancing in-edges."""
    import heapq

    order = np.argsort(-in_deg, kind="stable")
    heap = [(0, t) for t in range(NT)]
    heapq.heapify(heap)
    counts = np.zeros(NT, np.int32)
    newpos = np.empty(NPAD, np.int64)
    for v in order:
        load, t = heapq.heappop(heap)
        newpos[v] = t * P + counts[t]
        counts[t] += 1
        if counts[t] < P:
            heapq.heappush(heap, (load + int(in_deg[v]), t))
    return newpos


def _window_split(s2_t):
    """Split one tile's edge src ids into 3 windows; returns list of 3 arrays
    of edge positions (indices into s2_t)."""
    z = s2_t
    hard0 = z < WBASE[1]
    flex01 = (z >= WBASE[1]) & (z < WCAP)
    hard1 = (z >= WCAP) & (z < WBASE[2])
    flex12 = (z >= WBASE[2]) & (z < WBASE[1] + WCAP)
    hard2 = z >= WBASE[1] + WCAP
    n = len(z)
    tgt = n / 3.0
    n0, n4 = int(hard0.sum()), int(hard2.sum())
    a = int(np.clip(round(tgt - n0), 0, int(flex01.sum())))
    c = int(np.clip(round(tgt - n4), 0, int(flex12.sum())))
    i_f01 = np.nonzero(flex01)[0]
    i_f12 = np.nonzero(flex12)[0]
    w0 = np.concatenate([np.nonzero(hard0)[0], i_f01[:a]])
    w1 = np.concatenate([i_f01[a:], np.nonzero(hard1)[0], i_f12[c:]])
    w2 = np.concatenate([np.nonzero(hard2)[0], i_f12[:c]])
    return [w0, w1, w2]


def prep(x, edge_index, edge_weight):
    src = edge_index[0].astype(np.int64)
    dst = edge_index[1].astype(np.int64)
    ew = edge_weight.astype(np.float32)

    cnt = np.bincount(dst, minlength=N).astype(np.float32)
    wn = ew / np.maximum(cnt[dst], 1.0)

    in_deg = np.zeros(NPAD, np.int64)
    in_deg[:N] = np.bincount(dst, minlength=N)
    newpos = _assign_tiles(in_deg)

    s2 = newpos[src]
    d2 = newpos[dst]
    tile_of = d2 // P
    dl = (d2 % P).astype(np.float32)

    # per-tile edge lists
    order = np.argsort(tile_of, kind="stable")
    s2o, dlo, wno, tso = s2[order], dl[order], wn[order], tile_of[order]
    starts = np.searchsorted(tso, np.arange(NT + 1))

    # first pass: window split per tile, find quotas
    tile_windows = []
    bucket_sizes = np.zeros((NT, 3), np.int64)
    for t in range(NT):
        lo, hi = starts[t], starts[t + 1]
        wsplit = _window_split(s2o[lo:hi])
        tile_windows.append(wsplit)
        for w in range(3):
            bucket_sizes[t, w] = len(wsplit[w])
    Kw = [int(np.ceil(bucket_sizes[:, w].max() / P)) for w in range(3)]
    Kw = [max(k, 1) for k in Kw]
    K = sum(Kw)
    offw = [0, Kw[0], Kw[0] + Kw[1]]

    # second pass: pack slots; build one-hot weighted selection matrices S
    # S[t, c, p, d] = wn for the edge in (chunk c, partition p) with dst slot d
    s_full = np.zeros((NT, K, P, P), np.float16)
    # gather index lists: [NT, 3] ragged -> per (tile, w): int16 [Kw*P]
    gl = [np.zeros((NT, Kw[w] * P), np.int16) for w in range(3)]
    for t in range(NT):
        lo = starts[t]
        for w in range(3):
            pos = tile_windows[t][w]
            nw = len(pos)
            # sort by src index for ascending-address gather DMA
            pos = pos[np.argsort(s2o[lo + pos], kind="stable")]
            idxs = (s2o[lo + pos] - WBASE[w]).astype(np.int16)
            assert (idxs >= 0).all() and (idxs < WCAP).all()
            gl[w][t, :nw] = idxs
            sl = np.arange(nw)
            ch = sl // P
            pp = sl % P
            s_full[t, offw[w] + ch, pp, dlo[lo + pos].astype(np.int64)] = (
                wno[lo + pos]
            )

    # wrap gather lists into per-(group, window) 16-partition layout
    GI_COLS = G * K * 8
    gidx = np.zeros((NT // G, P, GI_COLS), np.int16)
    for g in range(NT // G):
        col = 0
        for w in range(3):
            seg = gl[w][g * G:(g + 1) * G].reshape(-1)  # [G*Kw*P]
            wrapped = seg.reshape(-1, 16).T  # [16, G*Kw*8]
            gidx[g, :, col:col + wrapped.shape[1]] = np.tile(wrapped, (8, 1))
            col += wrapped.shape[1]

    # transposed, padded, permuted node features
    xT = np.zeros((FPAD, NPAD), np.float16)
    xT[:IN_F, newpos[:N]] = x.T

    return {
        "newpos": newpos,
        "K": K,
        "Kw": tuple(Kw),
        "xT": xT,
        "sd": s_full,
        "gidx": gidx,
    }


def pack_weights(W1, b1, W2, b2, W3, b3, Wl, bl):
    def chunk_rhs(W, kchunks, dtype):
        # [F, 512] -> [128, kchunks, 512]
        F = W.shape[0]
        Wp = np.zeros((kchunks * P, 512), np.float32)
        Wp[:F] = W
        return np.ascontiguousarray(
            Wp.reshape(kchunks, P, 512).transpose(1, 0, 2)
        ).astype(dtype)

    w1cat = np.concatenate([W1[:IN_F], W1[IN_F:]], axis=1)  # [500, 512]
    w2cat = np.concatenate([W2[:HID], W2[HID:]], axis=1)  # [256, 512]
    w3cat = np.concatenate([W3[:HID], W3[HID:]], axis=1)
    wl = np.ascontiguousarray(
        Wl.reshape(6, P, NCLS).transpose(1, 0, 2)
    ).astype(np.float16)  # [128, 6, 7]
    return {
        "w1": chunk_rhs(w1cat, 4, np.float16),
        "w2": chunk_rhs(w2cat, 2, np.float16),
        "w3": chunk_rhs(w3cat, 2, np.float16),
        "wl": wl,
        "b1": np.tile(b1[None, :], (P, 1)).astype(np.float32),
        "b2": np.tile(b2[None, :], (P, 1)).astype(np.float32),
        "b3": np.tile(b3[None, :], (P, 1)).astype(np.float32),
        "bl": np.tile(bl[None, :], (P, 1)).astype(np.float32),
    }


# --------------------------------------------------------------------------
# device kernel
# --------------------------------------------------------------------------

def build(K, Kw, stage="full", repeat=1):
    """stage: 'a1' (phase A L1), 'ag1' (+AllGather), 'b1' (+phase B L1),
    'l2' (2 layers), 'full'.  repeat: run the whole pipeline N times
    (for differential timing)."""
    GI_COLS = G * K * 8
    offw = [0, Kw[0], Kw[0] + Kw[1]]

    nc = bacc.Bacc(
        "TRN2", target_bir_lowering=False, debug=False, num_devices=NC,
        num_swdge_queues=4,
    )

    xTc = nc.dram_tensor("xTc", [FPAD, NPC], f16, kind="ExternalInput")
    sd_d = nc.dram_tensor("sd", [TPC, K, P, P], f16, kind="ExternalInput")
    gidx_d = nc.dram_tensor("gidx", [NGRP, P, GI_COLS], i16, kind="ExternalInput")
    w1_d = nc.dram_tensor("w1", [P, 4, 512], f16, kind="ExternalInput")
    w2_d = nc.dram_tensor("w2", [P, 2, 512], f16, kind="ExternalInput")
    w3_d = nc.dram_tensor("w3", [P, 2, 512], f16, kind="ExternalInput")
    wl_d = nc.dram_tensor("wl", [P, 6, NCLS], f16, kind="ExternalInput")
    b1_d = nc.dram_tensor("b1", [P, HID], f32, kind="ExternalInput")
    b2_d = nc.dram_tensor("b2", [P, HID], f32, kind="ExternalInput")
    b3_d = nc.dram_tensor("b3", [P, HID], f32, kind="ExternalInput")
    bl_d = nc.dram_tensor("bl", [P, NCLS], f32, kind="ExternalInput")
    out_d = nc.dram_tensor("out", [NPC, NCLS], f32, kind="ExternalOutput")

    with tile.TileContext(nc) as tc:
        with (
            tc.tile_pool(name="dram", bufs=1, space="DRAM") as dram,
            tc.tile_pool(name="const", bufs=1) as cpool,
            tc.tile_pool(name="lx", bufs=6) as lxpool,
            tc.tile_pool(name="stage", bufs=3) as stpool,
            tc.tile_pool(name="msg", bufs=4) as msgpool,
            tc.tile_pool(name="sbuild", bufs=2) as sbpool,
            tc.tile_pool(name="psa", bufs=2, space="PSUM") as psa,
            tc.tile_pool(name="psagg", bufs=2, space="PSUM") as psagg,
            tc.tile_pool(name="pstr", bufs=2, space="PSUM") as pstr,
            tc.tile_pool(name="pscls", bufs=2, space="PSUM") as pscls,
        ):
            # ---- DRAM intermediates (allocated per repeat)
            def alloc_inter(rep):
                pl = [dram.tile([NPC, HID], f16, name=f"p{i}loc{rep}")
                      for i in range(3)]
                pf = [dram.tile([NPAD, HID], f16, addr_space="Shared",
                                name=f"p{i}full{rep}") for i in range(3)]
                rd = [dram.tile([NPC, HID], f16, name=f"r{i}d{rep}")
                      for i in range(3)]
                x2 = dram.tile([TPC, 2, P, P], f16, name=f"x2td{rep}")
                return pl, pf, rd, x2

            p_loc, p_full, r_dram, x2t_dram = alloc_inter(0)

            # ---- constants
            w1_sb = cpool.tile([P, 4, 512], f16)
            nc.sync.dma_start(w1_sb[:], w1_d[:])
            w2_sb = cpool.tile([P, 2, 512], f16)
            nc.sync.dma_start(w2_sb[:], w2_d[:])
            w3_sb = cpool.tile([P, 2, 512], f16)
            nc.sync.dma_start(w3_sb[:], w3_d[:])
            wl_sb = cpool.tile([P, 6, NCLS], f16)
            nc.sync.dma_start(wl_sb[:], wl_d[:])
            b_sb = []
            for name, t in (("b1", b1_d), ("b2", b2_d), ("b3", b3_d)):
                bt = cpool.tile([P, HID], f32, name=name + "sb")
                nc.sync.dma_start(bt[:], t[:])
                b_sb.append(bt)
            bl_sb = cpool.tile([P, NCLS], f32)
            nc.sync.dma_start(bl_sb[:], bl_d[:])

            gidx_sb = cpool.tile([P, NGRP, GI_COLS], i16)
            nc.sync.dma_start(
                gidx_sb[:],
                gidx_d[:].rearrange("g p c -> p g c"),
            )

            ident = cpool.tile([P, P], f16)
            make_identity(nc, ident[:])

            x1t_sb = cpool.tile([P, TPC, 2, P], f16)
            out_sb = cpool.tile([P, TPC, NCLS], f32)

            # ---- phase A: r/p for one layer (batched per 4-tile group)
            def grp_rows(buf, g):
                return buf[g * G * P:(g + 1) * G * P, :].rearrange(
                    "(t p) c -> p t c", p=P
                )

            def grp_blocks(buf, g):
                return buf[g * G:(g + 1) * G].rearrange("t h p q -> p (t h) q")

            def phase_a(layer):
                for g in range(NGRP):
                    if layer == 0:
                        lxs = []
                        for k in range(4):
                            lx = lxpool.tile([P, G * P], f16, name="lx", tag="lx")
                            nc.sync.dma_start(
                                lx[:],
                                xTc[k * P:(k + 1) * P, g * G * P:(g + 1) * G * P],
                            )
                            lxs.append(lx)
                    elif layer == 2:
                        x2s = lxpool.tile([P, 2 * G, P], f16, name="x2s", tag="x2s")
                        nc.sync.dma_start(x2s[:], grp_blocks(x2t_dram, g))
                    rst = stpool.tile([P, G, HID], f16, name="rst", tag="rst")
                    pst = stpool.tile([P, G, HID], f16, name="pst", tag="pst")
                    for gt in range(G):
                        t = g * G + gt
                        ps = psa.tile([P, 512], f32, name="psA", tag="psA")
                        if layer == 0:
                            for k in range(4):
                                nc.tensor.matmul(
                                    out=ps[:], lhsT=lxs[k][:, gt * P:(gt + 1) * P],
                                    rhs=w1_sb[:, k, :],
                                    start=(k == 0), stop=(k == 3),
                                )
                        elif layer == 1:
                            for k in range(2):
                                nc.tensor.matmul(
                                    out=ps[:], lhsT=x1t_sb[:, t, k, :],
                                    rhs=w2_sb[:, k, :],
                                    start=(k == 0), stop=(k == 1),
                                )
                        else:
                            for k in range(2):
                                nc.tensor.matmul(
                                    out=ps[:], lhsT=x2s[:, gt * 2 + k, :],
                                    rhs=w3_sb[:, k, :],
                                    start=(k == 0), stop=(k == 1),
                                )
                        nc.vector.tensor_tensor(
                            out=rst[:, gt, :], in0=ps[:, :HID], in1=b_sb[layer][:],
                            op=mybir.AluOpType.add,
                        )
                        nc.vector.tensor_copy(pst[:, gt, :], ps[:, HID:])
                    nc.sync.dma_start(grp_rows(r_dram[layer], g), rst[:])
                    nc.sync.dma_start(grp_rows(p_loc[layer], g), pst[:])

            # ---- phase B: aggregate + relu (+ classifier logits on last layer)
            def phase_b(layer, sub="full"):
                last = layer == 2
                for g in range(NGRP):
                    msgs = []
                    col = 0
                    for w in range(3):
                        ncols = G * Kw[w] * 8
                        m = msgpool.tile(
                            [P, G * Kw[w], HID], f16, name=f"m{w}", tag=f"m{w}"
                        )
                        nc.gpsimd.dma_gather(
                            out_ap=m[:],
                            in_ap=p_full[layer][WBASE[w]:, :],
                            idxs_ap=gidx_sb[:, g, col:col + ncols],
                            num_idxs=G * Kw[w] * P,
                            num_idxs_reg=G * Kw[w] * P,
                            elem_size=HID,
                            single_packet=(G * Kw[w] * P <= 1024),
                            queue_num=(g * 3 + w) % 4,
                        )
                        msgs.append(m)
                        col += ncols
                    s_grp = sbpool.tile([P, G * K, P], f16, name="sgrp", tag="sgrp")
                    nc.sync.dma_start(
                        s_grp[:],
                        sd_d[g * G:(g + 1) * G].rearrange("t k p q -> p (t k) q"),
                    )
                    rst = stpool.tile([P, G, HID], f16, name="rl", tag="rl")
                    nc.sync.dma_start(rst[:], grp_rows(r_dram[layer], g))
                    if layer == 1:
                        x2w = stpool.tile([P, 2 * G, P], f16, name="x2w", tag="x2w")
                    if last:
                        x2c = lxpool.tile([P, 2 * G, P], f16, name="x2c", tag="x2c")
                        nc.sync.dma_start(x2c[:], grp_blocks(x2t_dram, g))
                    for gt in range(G):
                        t = g * G + gt
                        agg = psagg.tile([P, HID], f32, name="agg", tag="agg")
                        c = 0
                        for w in range(3):
                            for j in range(Kw[w]):
                                nc.tensor.matmul(
                                    out=agg[:],
                                    lhsT=s_grp[:, gt * K + c, :],
                                    rhs=msgs[w][:, gt * Kw[w] + j, :],
                                    start=(c == 0),
                                    stop=(c == K - 1),
                                )
                                c += 1
                        xsum = stpool.tile([P, HID], f16, name="xsum", tag="xsum")
                        nc.vector.tensor_tensor(
                            out=xsum[:], in0=agg[:], in1=rst[:, gt, :],
                            op=mybir.AluOpType.add,
                        )
                        xn = stpool.tile([P, HID], f16, name="xn", tag="xn")
                        nc.scalar.activation(
                            xn[:], xsum[:], mybir.ActivationFunctionType.Relu
                        )
                        x3t = []
                        for h in range(2):
                            tp = pstr.tile([P, P], f16, name="tp", tag="tp")
                            nc.tensor.transpose(
                                out=tp[:], in_=xn[:, h * P:(h + 1) * P],
                                identity=ident[:],
                            )
                            if layer == 0:
                                nc.vector.tensor_copy(x1t_sb[:, t, h, :], tp[:])
                            elif layer == 1:
                                nc.vector.tensor_copy(x2w[:, gt * 2 + h, :], tp[:])
                            else:
                                xt = stpool.tile([P, P], f16, name="x3t", tag="x3t")
                                nc.vector.tensor_copy(xt[:], tp[:])
                                x3t.append(xt)
                        if last:
                            # classifier logits: 6 k-chunks of 128
                            cls = pscls.tile([P, NCLS], f32, name="cls", tag="cls")
                            chunks = [
                                x1t_sb[:, t, 0, :], x1t_sb[:, t, 1, :],
                                x2c[:, gt * 2, :], x2c[:, gt * 2 + 1, :],
                                x3t[0][:], x3t[1][:],
                            ]
                            for kk in range(6):
                                nc.tensor.matmul(
                                    out=cls[:], lhsT=chunks[kk], rhs=wl_sb[:, kk, :],
                                    start=(kk == 0), stop=(kk == 5),
                                )
                            nc.vector.tensor_copy(out_sb[:, t, :], cls[:])
                    if layer == 1 and sub == "full":
                        nc.sync.dma_start(grp_blocks(x2t_dram, g), x2w[:])

            # ---- batched log-softmax epilogue over all tiles
            def epilogue():
                lg = cpool.tile([P, TPC, NCLS], f32)
                nc.vector.tensor_tensor(
                    out=lg[:], in0=out_sb[:],
                    in1=bl_sb[:].unsqueeze(1).broadcast_to([P, TPC, NCLS]),
                    op=mybir.AluOpType.add,
                )
                mx = cpool.tile([P, TPC], f32)
                nc.vector.tensor_reduce(
                    out=mx[:], in_=lg[:], axis=mybir.AxisListType.X,
                    op=mybir.AluOpType.max,
                )
                sh = cpool.tile([P, TPC, NCLS], f32)
                nc.vector.tensor_tensor(
                    out=sh[:], in0=lg[:],
                    in1=mx[:].unsqueeze(2).broadcast_to([P, TPC, NCLS]),
                    op=mybir.AluOpType.subtract,
                )
                ex = cpool.tile([P, TPC, NCLS], f32)
                nc.scalar.activation(
                    ex[:], sh[:], mybir.ActivationFunctionType.Exp
                )
                sm = cpool.tile([P, TPC], f32)
                nc.vector.tensor_reduce(
                    out=sm[:], in_=ex[:], axis=mybir.AxisListType.X,
                    op=mybir.AluOpType.add,
                )
                lsm = cpool.tile([P, TPC], f32)
                nc.scalar.activation(
                    lsm[:], sm[:], mybir.ActivationFunctionType.Ln
                )
                nc.vector.tensor_tensor(
                    out=out_sb[:], in0=sh[:],
                    in1=lsm[:].unsqueeze(2).broadcast_to([P, TPC, NCLS]),
                    op=mybir.AluOpType.subtract,
                )

            if stage != "full":
                nc.gpsimd.memset(out_sb[:], 0.0)
            nlayers = {"a1": 1, "ag1": 1, "b1": 1, "l2": 2}.get(stage, 3)
            bsub = {"b1g": "gather", "b1s": "sbuild", "b1m": "mm",
                    "bg3": "gather", "bs3": "sbuild", "bm3": "mm",
                    "b1r": "relu"}.get(stage, "full")
            do_ag = stage not in ("a1", "a3", "noag")
            do_b = stage not in ("a1", "a3", "ag1", "ag3")
            if not do_b or bsub != "full":
                # timing-only variants: initialize tensors phase B would write
                nc.gpsimd.memset(x1t_sb[:], 0.0)
                nc.sync.dma_start(
                    x2t_dram[:].rearrange("t h p q -> p (t h) q"),
                    x1t_sb[:].rearrange("p t h q -> p (t h) q"),
                )
            for _rep in range(repeat):
                if _rep > 0:
                    p_loc, p_full, r_dram, x2t_dram = alloc_inter(_rep)
                for layer in range(nlayers):
                    phase_a(layer)
                    if do_ag:
                        nc.gpsimd.collective_compute(
                            "AllGather",
                            mybir.AluOpType.bypass,
                            replica_groups=[list(range(NC))],
                            ins=[p_loc[layer].opt()],
                            outs=[p_full[layer].opt()],
                        )
                    if do_b:
                        phase_b(layer, sub=bsub)
                if nlayers == 3 and bsub == "full":
                    epilogue()

            nc.sync.dma_start(
                out_d[:].rearrange("(t p) j -> p t j", p=P), out_sb[:]
            )

    nc.compile()
    return nc


# --------------------------------------------------------------------------
# entry point
# --------------------------------------------------------------------------

def kernel(x, edge_index, edge_weight, W1, b1, W2, b2, W3, b3, Wl, bl):
    x = np.asarray(x, dtype=np.float32)
    edge_index = np.asarray(edge_index)
    edge_weight = np.asarray(edge_weight, dtype=np.float32)

    pp = prep(x, edge_index, edge_weight)
    K, Kw = pp["K"], pp["Kw"]
    wts = pack_weights(
        np.asarray(W1, np.float32), np.asarray(b1, np.float32),
        np.asarray(W2, np.float32), np.asarray(b2, np.float32),
        np.asarray(W3, np.float32), np.asarray(b3, np.float32),
        np.asarray(Wl, np.float32), np.asarray(bl, np.float32),
    )

    key = (K, Kw)
    if key not in _compile_cache:
        _compile_cache[key] = build(K, Kw)
    nc = _compile_cache[key]

    in_maps = []
    for c in range(NC):
        in_maps.append({
            "xTc": np.ascontiguousarray(pp["xT"][:, c * NPC:(c + 1) * NPC]),
            "sd": np.ascontiguousarray(pp["sd"][c * TPC:(c + 1) * TPC]),
            "gidx": np.ascontiguousarray(pp["gidx"][c * NGRP:(c + 1) * NGRP]),
            **wts,
        })

    res = run_bass_kernel_spmd(nc, in_maps, list(range(NC)))
    out_full = np.concatenate([res.results[c]["out"] for c in range(NC)], axis=0)
    return out_full[pp["newpos"][:N]].astype(np.float32)


if __name__ == "__main__":
    import time

    rng = np.random.default_rng(0)
    # tiny self-check of prep packing invariants on random data
    E = 899756
    ei = rng.integers(0, N, (2, E)).astype(np.int32)
    ew = rng.random(E, dtype=np.float32)
    x = rng.standard_normal((N, IN_F), dtype=np.float32)
    t0 = time.time()
    pp = prep(x, ei, ew)
    print("prep", time.time() - t0, "K =", pp["K"], "Kw =", pp["Kw"])



# revision 17
# speedup vs baseline: 1.1045x; 1.1045x over previous
"""3-layer GraphSAGE + classifier + log_softmax on 8 Trainium2 NeuronCores.

Self-contained: host-side sharding/packing + Bass/Tile device kernel.

Strategy
--------
concat([x, agg]) @ W  ==  x @ W_top + Ahat @ (x @ W_bot)   (linearity)
so aggregation happens in the 256-dim projected space.

- Nodes are permuted into 704 tiles of 128 (in-degree balanced), 88 tiles/core.
- Per layer: phase A computes r = x@W_top + b and p = x@W_bot per owned tile
  (PE matmuls, fp32r for layer 1, fp16 after), p is written fp16 and
  AllGathered so every core holds the full p table in DRAM.
- Phase B: per dst tile, gather p[src] rows for its in-edges via dma_gather
  (int16 indices -> three overlapping 32768-row windows), build a one-hot
  selection matrix S[e, d] = wn_e * (dst_local_e == d) on DVE, and accumulate
  agg = sum_c S_c.T @ msg_c on the PE into PSUM.  x_next = relu(agg + r).
- x_next is transposed on the PE (2x 128x128) to feed the next layer's
  stationary operand; the classifier (768->7) + log_softmax are fused into
  layer-3 phase B.
"""

import numpy as np

import concourse.bass as bass
import concourse.mybir as mybir
import concourse.tile as tile
from concourse import bacc
from concourse.bass_utils import run_bass_kernel_spmd
from concourse.masks import make_identity

# problem constants
N = 89250
IN_F = 500
HID = 256
NCLS = 7
FPAD = 512  # padded input feature dim

NC = 8  # cores
P = 128
NT = 704  # node tiles
TPC = NT // NC  # 88 tiles per core
NPAD = NT * P  # 90112
NPC = TPC * P  # 11264 nodes per core
G = 2  # tiles per gather group
NGRP = TPC // G

WBASE = (0, 28672, 57344)  # gather window base rows
WCAP = 32768  # int16 index reach

f32 = mybir.dt.float32
f32r = mybir.dt.float32r
f16 = mybir.dt.float16
i16 = mybir.dt.int16
i32 = mybir.dt.int32

_compile_cache = {}


# --------------------------------------------------------------------------
# host-side prep
# --------------------------------------------------------------------------

def _assign_tiles(in_deg):
    """LPT: assign node ids (0..NPAD) to (tile, slot), balancing in-edges."""
    import heapq

    order = np.argsort(-in_deg, kind="stable")
    heap = [(0, t) for t in range(NT)]
    heapq.heapify(heap)
    counts = np.zeros(NT, np.int32)
    newpos = np.empty(NPAD, np.int64)
    for v in order:
        load, t = heapq.heappop(heap)
        newpos[v] = t * P + counts[t]
        counts[t] += 1
        if counts[t] < P:
            heapq.heappush(heap, (load + int(in_deg[v]), t))
    return newpos


def _window_split(s2_t):
    """Split one tile's edge src ids into 3 windows; returns list of 3 arrays
    of edge positions (indices into s2_t)."""
    z = s2_t
    hard0 = z < WBASE[1]
    flex01 = (z >= WBASE[1]) & (z < WCAP)
    hard1 = (z >= WCAP) & (z < WBASE[2])
    flex12 = (z >= WBASE[2]) & (z < WBASE[1] + WCAP)
    hard2 = z >= WBASE[1] + WCAP
    n = len(z)
    tgt = n / 3.0
    n0, n4 = int(hard0.sum()), int(hard2.sum())
    a = int(np.clip(round(tgt - n0), 0, int(flex01.sum())))
    c = int(np.clip(round(tgt - n4), 0, int(flex12.sum())))
    i_f01 = np.nonzero(flex01)[0]
    i_f12 = np.nonzero(flex12)[0]
    w0 = np.concatenate([np.nonzero(hard0)[0], i_f01[:a]])
    w1 = np.concatenate([i_f01[a:], np.nonzero(hard1)[0], i_f12[c:]])
    w2 = np.concatenate([np.nonzero(hard2)[0], i_f12[:c]])
    return [w0, w1, w2]


def prep(x, edge_index, edge_weight):
    src = edge_index[0].astype(np.int64)
    dst = edge_index[1].astype(np.int64)
    ew = edge_weight.astype(np.float32)

    cnt = np.bincount(dst, minlength=N).astype(np.float32)
    wn = ew / np.maximum(cnt[dst], 1.0)

    in_deg = np.zeros(NPAD, np.int64)
    in_deg[:N] = np.bincount(dst, minlength=N)
    newpos = _assign_tiles(in_deg)

    s2 = newpos[src]
    d2 = newpos[dst]
    tile_of = d2 // P
    dl = (d2 % P).astype(np.float32)

    # per-tile edge lists
    order = np.argsort(tile_of, kind="stable")
    s2o, dlo, wno, tso = s2[order], dl[order], wn[order], tile_of[order]
    starts = np.searchsorted(tso, np.arange(NT + 1))

    # first pass: window split per tile, find quotas
    tile_windows = []
    bucket_sizes = np.zeros((NT, 3), np.int64)
    for t in range(NT):
        lo, hi = starts[t], starts[t + 1]
        wsplit = _window_split(s2o[lo:hi])
        tile_windows.append(wsplit)
        for w in range(3):
            bucket_sizes[t, w] = len(wsplit[w])
    Kw = [int(np.ceil(bucket_sizes[:, w].max() / P)) for w in range(3)]
    Kw = [max(k, 1) for k in Kw]
    K = sum(Kw)
    offw = [0, Kw[0], Kw[0] + Kw[1]]

    # second pass: pack slots; build one-hot weighted selection matrices S
    # S[t, p, c, d] = wn for the edge in (chunk c, partition p) with dst slot d
    # (partition-major so the device DMA reads contiguous K*P runs)
    s_full = np.zeros((NT, P, K, P), np.float16)
    # gather index lists: [NT, 3] ragged -> per (tile, w): int16 [Kw*P]
    gl = [np.zeros((NT, Kw[w] * P), np.int16) for w in range(3)]
    for t in range(NT):
        lo = starts[t]
        for w in range(3):
            pos = tile_windows[t][w]
            nw = len(pos)
            # sort by src index for ascending-address gather DMA
            pos = pos[np.argsort(s2o[lo + pos], kind="stable")]
            idxs = (s2o[lo + pos] - WBASE[w]).astype(np.int16)
            assert (idxs >= 0).all() and (idxs < WCAP).all()
            gl[w][t, :nw] = idxs
            sl = np.arange(nw)
            ch = sl // P
            pp = sl % P
            s_full[t, pp, offw[w] + ch, dlo[lo + pos].astype(np.int64)] = (
                wno[lo + pos]
            )

    # wrap gather lists into per-(group, window) 16-partition layout
    GI_COLS = G * K * 8
    gidx = np.zeros((NT // G, P, GI_COLS), np.int16)
    for g in range(NT // G):
        col = 0
        for w in range(3):
            seg = gl[w][g * G:(g + 1) * G].reshape(-1)  # [G*Kw*P]
            wrapped = seg.reshape(-1, 16).T  # [16, G*Kw*8]
            gidx[g, :, col:col + wrapped.shape[1]] = np.tile(wrapped, (8, 1))
            col += wrapped.shape[1]

    # transposed, padded, permuted node features
    xT = np.zeros((FPAD, NPAD), np.float16)
    xT[:IN_F, newpos[:N]] = x.T

    return {
        "newpos": newpos,
        "K": K,
        "Kw": tuple(Kw),
        "xT": xT,
        "sd": s_full,
        "gidx": gidx,
    }


def pack_weights(W1, b1, W2, b2, W3, b3, Wl, bl):
    def chunk_rhs(W, kchunks, dtype):
        # [F, 512] -> [128, kchunks, 512]
        F = W.shape[0]
        Wp = np.zeros((kchunks * P, 512), np.float32)
        Wp[:F] = W
        return np.ascontiguousarray(
            Wp.reshape(kchunks, P, 512).transpose(1, 0, 2)
        ).astype(dtype)

    w1cat = np.concatenate([W1[:IN_F], W1[IN_F:]], axis=1)  # [500, 512]
    w2cat = np.concatenate([W2[:HID], W2[HID:]], axis=1)  # [256, 512]
    w3cat = np.concatenate([W3[:HID], W3[HID:]], axis=1)
    wl = np.ascontiguousarray(
        Wl.reshape(6, P, NCLS).transpose(1, 0, 2)
    ).astype(np.float16)  # [128, 6, 7]
    return {
        "w1": chunk_rhs(w1cat, 4, np.float16),
        "w2": chunk_rhs(w2cat, 2, np.float16),
        "w3": chunk_rhs(w3cat, 2, np.float16),
        "wl": wl,
        "b1": np.tile(b1[None, :], (P, 1)).astype(np.float32),
        "b2": np.tile(b2[None, :], (P, 1)).astype(np.float32),
        "b3": np.tile(b3[None, :], (P, 1)).astype(np.float32),
        "bl": np.tile(bl[None, :], (P, 1)).astype(np.float32),
    }


# --------------------------------------------------------------------------
# device kernel
# --------------------------------------------------------------------------

def build(K, Kw, stage="full", repeat=1):
    """stage: 'a1' (phase A L1), 'ag1' (+AllGather), 'b1' (+phase B L1),
    'l2' (2 layers), 'full'.  repeat: run the whole pipeline N times
    (for differential timing)."""
    GI_COLS = G * K * 8
    offw = [0, Kw[0], Kw[0] + Kw[1]]

    nc = bacc.Bacc(
        "TRN2", target_bir_lowering=False, debug=False, num_devices=NC,
        num_swdge_queues=4,
    )

    xTc = nc.dram_tensor("xTc", [FPAD, NPC], f16, kind="ExternalInput")
    sd_d = nc.dram_tensor("sd", [TPC, P, K, P], f16, kind="ExternalInput")
    gidx_d = nc.dram_tensor("gidx", [NGRP, P, GI_COLS], i16, kind="ExternalInput")
    w1_d = nc.dram_tensor("w1", [P, 4, 512], f16, kind="ExternalInput")
    w2_d = nc.dram_tensor("w2", [P, 2, 512], f16, kind="ExternalInput")
    w3_d = nc.dram_tensor("w3", [P, 2, 512], f16, kind="ExternalInput")
    wl_d = nc.dram_tensor("wl", [P, 6, NCLS], f16, kind="ExternalInput")
    b1_d = nc.dram_tensor("b1", [P, HID], f32, kind="ExternalInput")
    b2_d = nc.dram_tensor("b2", [P, HID], f32, kind="ExternalInput")
    b3_d = nc.dram_tensor("b3", [P, HID], f32, kind="ExternalInput")
    bl_d = nc.dram_tensor("bl", [P, NCLS], f32, kind="ExternalInput")
    out_d = nc.dram_tensor("out", [NPC, NCLS], f32, kind="ExternalOutput")

    with tile.TileContext(nc) as tc:
        with (
            tc.tile_pool(name="dram", bufs=1, space="DRAM") as dram,
            tc.tile_pool(name="const", bufs=1) as cpool,
            tc.tile_pool(name="lx", bufs=6) as lxpool,
            tc.tile_pool(name="stage", bufs=3) as stpool,
            tc.tile_pool(name="msg", bufs=4) as msgpool,
            tc.tile_pool(name="sbuild", bufs=2) as sbpool,
            tc.tile_pool(name="psa", bufs=2, space="PSUM") as psa,
            tc.tile_pool(name="psagg", bufs=2, space="PSUM") as psagg,
            tc.tile_pool(name="pstr", bufs=2, space="PSUM") as pstr,
            tc.tile_pool(name="pscls", bufs=2, space="PSUM") as pscls,
        ):
            # ---- DRAM intermediates (allocated per repeat)
            def alloc_inter(rep):
                pl = [dram.tile([NPC, HID], f16, name=f"p{i}loc{rep}")
                      for i in range(3)]
                pf = [dram.tile([NPAD, HID], f16, addr_space="Shared",
                                name=f"p{i}full{rep}") for i in range(3)]
                rd = [dram.tile([NPC, HID], f16, name=f"r{i}d{rep}")
                      for i in range(3)]
                x2 = dram.tile([TPC, 2, P, P], f16, name=f"x2td{rep}")
                return pl, pf, rd, x2

            p_loc, p_full, r_dram, x2t_dram = alloc_inter(0)

            # ---- constants
            w1_sb = cpool.tile([P, 4, 512], f16)
            nc.sync.dma_start(w1_sb[:], w1_d[:])
            w2_sb = cpool.tile([P, 2, 512], f16)
            nc.sync.dma_start(w2_sb[:], w2_d[:])
            w3_sb = cpool.tile([P, 2, 512], f16)
            nc.sync.dma_start(w3_sb[:], w3_d[:])
            wl_sb = cpool.tile([P, 6, NCLS], f16)
            nc.sync.dma_start(wl_sb[:], wl_d[:])
            b_sb = []
            for name, t in (("b1", b1_d), ("b2", b2_d), ("b3", b3_d)):
                bt = cpool.tile([P, HID], f32, name=name + "sb")
                nc.sync.dma_start(bt[:], t[:])
                b_sb.append(bt)
            bl_sb = cpool.tile([P, NCLS], f32)
            nc.sync.dma_start(bl_sb[:], bl_d[:])

            gidx_sb = cpool.tile([P, NGRP, GI_COLS], i16)
            nc.sync.dma_start(
                gidx_sb[:],
                gidx_d[:].rearrange("g p c -> p g c"),
            )

            ident = cpool.tile([P, P], f16)
            make_identity(nc, ident[:])

            x1t_sb = cpool.tile([P, TPC, 2, P], f16)
            out_sb = cpool.tile([P, TPC, NCLS], f32)

            # ---- phase A: r/p for one layer (batched per 4-tile group)
            def grp_rows(buf, g):
                return buf[g * G * P:(g + 1) * G * P, :].rearrange(
                    "(t p) c -> p t c", p=P
                )

            def grp_blocks(buf, g):
                return buf[g * G:(g + 1) * G].rearrange("t h p q -> p (t h) q")

            def phase_a(layer):
                for g in range(NGRP):
                    if layer == 0:
                        lxs = []
                        for k in range(4):
                            lx = lxpool.tile([P, G * P], f16, name="lx", tag="lx")
                            nc.sync.dma_start(
                                lx[:],
                                xTc[k * P:(k + 1) * P, g * G * P:(g + 1) * G * P],
                            )
                            lxs.append(lx)
                    elif layer == 2:
                        x2s = lxpool.tile([P, 2 * G, P], f16, name="x2s", tag="x2s")
                        nc.sync.dma_start(x2s[:], grp_blocks(x2t_dram, g))
                    rst = stpool.tile([P, G, HID], f16, name="rst", tag="rst")
                    pst = stpool.tile([P, G, HID], f16, name="pst", tag="pst")
                    for gt in range(G):
                        t = g * G + gt
                        ps = psa.tile([P, 512], f32, name="psA", tag="psA")
                        if layer == 0:
                            for k in range(4):
                                nc.tensor.matmul(
                                    out=ps[:], lhsT=lxs[k][:, gt * P:(gt + 1) * P],
                                    rhs=w1_sb[:, k, :],
                                    start=(k == 0), stop=(k == 3),
                                )
                        elif layer == 1:
                            for k in range(2):
                                nc.tensor.matmul(
                                    out=ps[:], lhsT=x1t_sb[:, t, k, :],
                                    rhs=w2_sb[:, k, :],
                                    start=(k == 0), stop=(k == 1),
                                )
                        else:
                            for k in range(2):
                                nc.tensor.matmul(
                                    out=ps[:], lhsT=x2s[:, gt * 2 + k, :],
                                    rhs=w3_sb[:, k, :],
                                    start=(k == 0), stop=(k == 1),
                                )
                        nc.vector.tensor_tensor(
                            out=rst[:, gt, :], in0=ps[:, :HID], in1=b_sb[layer][:],
                            op=mybir.AluOpType.add,
                        )
                        nc.vector.tensor_copy(pst[:, gt, :], ps[:, HID:])
                    nc.sync.dma_start(grp_rows(r_dram[layer], g), rst[:])
                    nc.sync.dma_start(grp_rows(p_loc[layer], g), pst[:])

            # ---- phase B: aggregate + relu (+ classifier logits on last layer)
            def phase_b(layer, sub="full"):
                last = layer == 2
                for g in range(NGRP):
                    msgs = []
                    col = 0
                    for w in range(3):
                        ncols = G * Kw[w] * 8
                        m = msgpool.tile(
                            [P, G * Kw[w], HID], f16, name=f"m{w}", tag=f"m{w}"
                        )
                        nc.gpsimd.dma_gather(
                            out_ap=m[:],
                            in_ap=p_full[layer][WBASE[w]:, :],
                            idxs_ap=gidx_sb[:, g, col:col + ncols],
                            num_idxs=G * Kw[w] * P,
                            num_idxs_reg=G * Kw[w] * P,
                            elem_size=HID,
                            single_packet=(G * Kw[w] * P <= 1024),
                            queue_num=(g * 3 + w) % 4,
                        )
                        msgs.append(m)
                        col += ncols
                    s_grp = sbpool.tile([P, G, K, P], f16, name="sgrp", tag="sgrp")
                    nc.sync.dma_start(
                        s_grp[:],
                        sd_d[g * G:(g + 1) * G].rearrange("t p k q -> p t k q"),
                    )
                    rst = stpool.tile([P, G, HID], f16, name="rl", tag="rl")
                    nc.sync.dma_start(rst[:], grp_rows(r_dram[layer], g))
                    if layer == 1:
                        x2w = stpool.tile([P, 2 * G, P], f16, name="x2w", tag="x2w")
                    if last:
                        x2c = lxpool.tile([P, 2 * G, P], f16, name="x2c", tag="x2c")
                        nc.sync.dma_start(x2c[:], grp_blocks(x2t_dram, g))
                    for gt in range(G):
                        t = g * G + gt
                        agg = psagg.tile([P, HID], f32, name="agg", tag="agg")
                        c = 0
                        for w in range(3):
                            for j in range(Kw[w]):
                                nc.tensor.matmul(
                                    out=agg[:],
                                    lhsT=s_grp[:, gt, c, :],
                                    rhs=msgs[w][:, gt * Kw[w] + j, :],
                                    start=(c == 0),
                                    stop=(c == K - 1),
                                )
                                c += 1
                        xsum = stpool.tile([P, HID], f16, name="xsum", tag="xsum")
                        nc.vector.tensor_tensor(
                            out=xsum[:], in0=agg[:], in1=rst[:, gt, :],
                            op=mybir.AluOpType.add,
                        )
                        xn = stpool.tile([P, HID], f16, name="xn", tag="xn")
                        nc.scalar.activation(
                            xn[:], xsum[:], mybir.ActivationFunctionType.Relu
                        )
                        x3t = []
                        for h in range(2):
                            tp = pstr.tile([P, P], f16, name="tp", tag="tp")
                            nc.tensor.transpose(
                                out=tp[:], in_=xn[:, h * P:(h + 1) * P],
                                identity=ident[:],
                            )
                            if layer == 0:
                                nc.vector.tensor_copy(x1t_sb[:, t, h, :], tp[:])
                            elif layer == 1:
                                nc.vector.tensor_copy(x2w[:, gt * 2 + h, :], tp[:])
                            else:
                                xt = stpool.tile([P, P], f16, name="x3t", tag="x3t")
                                nc.vector.tensor_copy(xt[:], tp[:])
                                x3t.append(xt)
                        if last:
                            # classifier logits: 6 k-chunks of 128
                            cls = pscls.tile([P, NCLS], f32, name="cls", tag="cls")
                            chunks = [
                                x1t_sb[:, t, 0, :], x1t_sb[:, t, 1, :],
                                x2c[:, gt * 2, :], x2c[:, gt * 2 + 1, :],
                                x3t[0][:], x3t[1][:],
                            ]
                            for kk in range(6):
                                nc.tensor.matmul(
                                    out=cls[:], lhsT=chunks[kk], rhs=wl_sb[:, kk, :],
                                    start=(kk == 0), stop=(kk == 5),
                                )
                            nc.vector.tensor_copy(out_sb[:, t, :], cls[:])
                    if layer == 1 and sub == "full":
                        nc.sync.dma_start(grp_blocks(x2t_dram, g), x2w[:])

            # ---- batched log-softmax epilogue over all tiles
            def epilogue():
                lg = cpool.tile([P, TPC, NCLS], f32)
                nc.vector.tensor_tensor(
                    out=lg[:], in0=out_sb[:],
                    in1=bl_sb[:].unsqueeze(1).broadcast_to([P, TPC, NCLS]),
                    op=mybir.AluOpType.add,
                )
                mx = cpool.tile([P, TPC], f32)
                nc.vector.tensor_reduce(
                    out=mx[:], in_=lg[:], axis=mybir.AxisListType.X,
                    op=mybir.AluOpType.max,
                )
                sh = cpool.tile([P, TPC, NCLS], f32)
                nc.vector.tensor_tensor(
                    out=sh[:], in0=lg[:],
                    in1=mx[:].unsqueeze(2).broadcast_to([P, TPC, NCLS]),
                    op=mybir.AluOpType.subtract,
                )
                ex = cpool.tile([P, TPC, NCLS], f32)
                nc.scalar.activation(
                    ex[:], sh[:], mybir.ActivationFunctionType.Exp
                )
                sm = cpool.tile([P, TPC], f32)
                nc.vector.tensor_reduce(
                    out=sm[:], in_=ex[:], axis=mybir.AxisListType.X,
                    op=mybir.AluOpType.add,
                )
                lsm = cpool.tile([P, TPC], f32)
                nc.scalar.activation(
                    lsm[:], sm[:], mybir.ActivationFunctionType.Ln
                )
                nc.vector.tensor_tensor(
                    out=out_sb[:], in0=sh[:],
                    in1=lsm[:].unsqueeze(2).broadcast_to([P, TPC, NCLS]),
                    op=mybir.AluOpType.subtract,
                )

            if stage != "full":
                nc.gpsimd.memset(out_sb[:], 0.0)
            nlayers = {"a1": 1, "ag1": 1, "b1": 1, "l2": 2}.get(stage, 3)
            bsub = {"b1g": "gather", "b1s": "sbuild", "b1m": "mm",
                    "bg3": "gather", "bs3": "sbuild", "bm3": "mm",
                    "b1r": "relu"}.get(stage, "full")
            do_ag = stage not in ("a1", "a3", "noag")
            do_b = stage not in ("a1", "a3", "ag1", "ag3")
            if not do_b or bsub != "full":
                # timing-only variants: initialize tensors phase B would write
                nc.gpsimd.memset(x1t_sb[:], 0.0)
                nc.sync.dma_start(
                    x2t_dram[:].rearrange("t h p q -> p (t h) q"),
                    x1t_sb[:].rearrange("p t h q -> p (t h) q"),
                )
            for _rep in range(repeat):
                if _rep > 0:
                    p_loc, p_full, r_dram, x2t_dram = alloc_inter(_rep)
                for layer in range(nlayers):
                    phase_a(layer)
                    if do_ag:
                        nc.gpsimd.collective_compute(
                            "AllGather",
                            mybir.AluOpType.bypass,
                            replica_groups=[list(range(NC))],
                            ins=[p_loc[layer].opt()],
                            outs=[p_full[layer].opt()],
                        )
                    if do_b:
                        phase_b(layer, sub=bsub)
                if nlayers == 3 and bsub == "full":
                    epilogue()

            nc.sync.dma_start(
                out_d[:].rearrange("(t p) j -> p t j", p=P), out_sb[:]
            )

    nc.compile()
    return nc


# --------------------------------------------------------------------------
# entry point
# --------------------------------------------------------------------------

def kernel(x, edge_index, edge_weight, W1, b1, W2, b2, W3, b3, Wl, bl):
    x = np.asarray(x, dtype=np.float32)
    edge_index = np.asarray(edge_index)
    edge_weight = np.asarray(edge_weight, dtype=np.float32)

    pp = prep(x, edge_index, edge_weight)
    K, Kw = pp["K"], pp["Kw"]
    wts = pack_weights(
        np.asarray(W1, np.float32), np.asarray(b1, np.float32),
        np.asarray(W2, np.float32), np.asarray(b2, np.float32),
        np.asarray(W3, np.float32), np.asarray(b3, np.float32),
        np.asarray(Wl, np.float32), np.asarray(bl, np.float32),
    )

    key = (K, Kw)
    if key not in _compile_cache:
        _compile_cache[key] = build(K, Kw)
    nc = _compile_cache[key]

    in_maps = []
    for c in range(NC):
        in_maps.append({
            "xTc": np.ascontiguousarray(pp["xT"][:, c * NPC:(c + 1) * NPC]),
            "sd": np.ascontiguousarray(pp["sd"][c * TPC:(c + 1) * TPC]),
            "gidx": np.ascontiguousarray(pp["gidx"][c * NGRP:(c + 1) * NGRP]),
            **wts,
        })

    res = run_bass_kernel_spmd(nc, in_maps, list(range(NC)))
    out_full = np.concatenate([res.results[c]["out"] for c in range(NC)], axis=0)
    return out_full[pp["newpos"][:N]].astype(np.float32)


if __name__ == "__main__":
    import time

    rng = np.random.default_rng(0)
    # tiny self-check of prep packing invariants on random data
    E = 899756
    ei = rng.integers(0, N, (2, E)).astype(np.int32)
    ew = rng.random(E, dtype=np.float32)
    x = rng.standard_normal((N, IN_F), dtype=np.float32)
    t0 = time.time()
    pp = prep(x, ei, ew)
    print("prep", time.time() - t0, "K =", pp["K"], "Kw =", pp["Kw"])



# revision 19
# speedup vs baseline: 1.2828x; 1.1614x over previous
"""3-layer GraphSAGE + classifier + log_softmax on 8 Trainium2 NeuronCores.

Self-contained: host-side sharding/packing + Bass/Tile device kernel.

Strategy
--------
concat([x, agg]) @ W  ==  x @ W_top + Ahat @ (x @ W_bot)   (linearity)
so aggregation happens in the 256-dim projected space.

- Nodes are permuted into 704 tiles of 128 (in-degree balanced), 88 tiles/core.
- Per layer: phase A computes r = x@W_top + b and p = x@W_bot per owned tile.
  p is written fp16 in two halves; each half is AllGathered into its own
  table (45056 rows) as soon as the half is computed, hiding collective
  latency under the remaining phase-A work.
- Phase B: per dst tile, gather p[src] rows for its in-edges via dma_gather
  (int16 indices; 4 overlapping windows, 2 per table). Window 0/2 carry a
  fixed 384 edges per tile (zero padding); windows 1/3 carry the remainder
  with trailing -1 indices that the gather ucode truncates per core.
  Gathers round-robin the 4 SWDGE queues so all 4 Q7 core pairs generate
  descriptors concurrently. The weighted one-hot selection matrices
  S[e, d] = wn_e * (dst_local_e == d) are precomputed on the host and
  DMAed; agg = sum_c S_c.T @ msg_c accumulates on the PE.
  x_next = relu(agg + r).
- x_next is transposed on the PE (2x 128x128) to feed the next layer's
  stationary operand; the classifier (768->7) runs per tile in layer-3
  phase B; the log_softmax runs batched (max/exp/sum per 4-tile block,
  single ln at the end).
"""

import numpy as np

import concourse.bass as bass
import concourse.mybir as mybir
import concourse.tile as tile
from concourse import bacc
from concourse.bass_utils import run_bass_kernel_spmd
from concourse.masks import make_identity

# problem constants
N = 89250
IN_F = 500
HID = 256
NCLS = 7
FPAD = 512  # padded input feature dim

NC = 8  # cores
P = 128
NT = 704  # node tiles
TPC = NT // NC  # 88 tiles per core
NPAD = NT * P  # 90112
NPC = TPC * P  # 11264 nodes per core
HT = TPC // 2  # 44 tiles per half
HROWS = HT * P  # 5632 rows per half per core
TAB = NC * HROWS  # 45056 rows per AllGathered table

WCAP = 32768  # int16 index reach
W1BASE = TAB - WCAP  # 12288: base row of windows 1/3 within their table
SPLIT0 = 384  # fixed edges per tile routed to window 0 (and window 2)

f32 = mybir.dt.float32
f16 = mybir.dt.float16
i16 = mybir.dt.int16
i32 = mybir.dt.int32

_compile_cache = {}


# --------------------------------------------------------------------------
# host-side prep
# --------------------------------------------------------------------------

def _assign_tiles(in_deg):
    """LPT: assign node ids (0..NPAD) to (tile, slot), balancing in-edges."""
    import heapq

    order = np.argsort(-in_deg, kind="stable")
    heap = [(0, t) for t in range(NT)]
    heapq.heapify(heap)
    counts = np.zeros(NT, np.int32)
    newpos = np.empty(NPAD, np.int64)
    for v in order:
        load, t = heapq.heappop(heap)
        newpos[v] = t * P + counts[t]
        counts[t] += 1
        if counts[t] < P:
            heapq.heappush(heap, (load + int(in_deg[v]), t))
    return newpos


def _ru16(x):
    return (int(x) + 15) // 16 * 16


def prep(x, edge_index, edge_weight):
    src = edge_index[0].astype(np.int64)
    dst = edge_index[1].astype(np.int64)
    ew = edge_weight.astype(np.float32)

    cnt = np.bincount(dst, minlength=N).astype(np.float32)
    wn = ew / np.maximum(cnt[dst], 1.0)

    in_deg = np.zeros(NPAD, np.int64)
    in_deg[:N] = np.bincount(dst, minlength=N)
    newpos = _assign_tiles(in_deg)

    s2 = newpos[src]
    d2 = newpos[dst]
    # table row of each source: owner core c, local row jl; half A = first
    # 44 tiles of the core, half B = rest.  trow = c*HROWS + (jl mod HROWS)
    c_own = s2 // NPC
    jl = s2 % NPC
    is_b = jl >= HROWS
    trow = c_own * HROWS + (jl - HROWS * is_b)

    tile_of = d2 // P
    dl = (d2 % P).astype(np.int64)

    order = np.argsort(tile_of, kind="stable")
    trow_o, isb_o, dl_o, wn_o = trow[order], is_b[order], dl[order], wn[order]
    tile_o = tile_of[order]
    starts = np.searchsorted(tile_o, np.arange(NT + 1))

    # per (tile, window): sorted index lists + (dl, wn) in slot order
    # windows: 0 = A[0:32768), 1 = A[12288:45056), 2/3 same for B
    tw_idx = [[None] * 4 for _ in range(NT)]
    tw_dl = [[None] * 4 for _ in range(NT)]
    tw_wn = [[None] * 4 for _ in range(NT)]
    for t in range(NT):
        lo, hi = starts[t], starts[t + 1]
        tr, ib = trow_o[lo:hi], isb_o[lo:hi]
        dd, ww = dl_o[lo:hi], wn_o[lo:hi]
        for half in range(2):
            sel = np.nonzero(ib == half)[0]
            o = sel[np.argsort(tr[sel], kind="stable")]
            n = len(o)
            assert n >= SPLIT0, f"tile {t} half {half}: only {n} edges"
            assert tr[o[SPLIT0 - 1]] < WCAP, f"tile {t}: w0 split infeasible"
            assert tr[o[SPLIT0]] >= W1BASE, f"tile {t}: w1 split infeasible"
            w0, w1 = 2 * half, 2 * half + 1
            tw_idx[t][w0] = tr[o[:SPLIT0]].astype(np.int16)
            tw_idx[t][w1] = (tr[o[SPLIT0:]] - W1BASE).astype(np.int16)
            tw_dl[t][w0], tw_dl[t][w1] = dd[o[:SPLIT0]], dd[o[SPLIT0:]]
            tw_wn[t][w0], tw_wn[t][w1] = ww[o[:SPLIT0]], ww[o[SPLIT0:]]

    # per tile-slot budgets (uniform across cores for SPMD)
    b16 = np.zeros((TPC, 4), np.int64)  # padded index counts
    for tl in range(TPC):
        for w in range(4):
            mx = max(len(tw_idx[c * TPC + tl][w]) for c in range(NC))
            b16[tl, w] = _ru16(mx)
    cb = (b16 + P - 1) // P  # chunk budgets
    kc = cb.sum(axis=1)  # chunks per tile
    soff = np.zeros(TPC + 1, np.int64)
    soff[1:] = np.cumsum(kc)
    sct = int(soff[-1])
    goff = np.zeros((TPC, 5), np.int64)
    for tl in range(TPC):
        goff[tl, 1:] = np.cumsum(b16[tl] // 16)
    gmax = int(goff[:, 4].max())

    # pack gidx [TPC, P, gmax] and S [NC][P, sct, P]
    gidx = np.full((NT, P, gmax), -1, np.int16)
    sd = np.zeros((NC, P, sct, P), np.float16)
    for t in range(NT):
        c, tl = t // TPC, t % TPC
        ci0 = 0
        for w in range(4):
            idx = tw_idx[t][w]
            n = len(idx)
            b = int(b16[tl, w])
            # pad with a repeat of the last index: full descriptor count
            # (num_idxs_reg accounting) at near-zero HBM cost (row-buffer hit)
            arr = np.full(b, idx[-1], np.int16)
            arr[:n] = idx
            wrapped = arr.reshape(-1, 16).T  # [16, b/16]
            gidx[t, :, goff[tl, w]:goff[tl, w + 1]] = np.tile(wrapped, (8, 1))
            sl = np.arange(n)
            sd[c, sl % P, soff[tl] + ci0 + sl // P, tw_dl[t][w]] = tw_wn[t][w]
            ci0 += int(cb[tl, w])

    # transposed, padded, permuted node features
    xT = np.zeros((FPAD, NPAD), np.float16)
    xT[:IN_F, newpos[:N]] = x.T

    return {
        "newpos": newpos,
        "b16": b16,
        "cb": cb,
        "kc": kc,
        "soff": soff,
        "goff": goff,
        "sct": sct,
        "gmax": gmax,
        "xT": xT,
        "sd": sd,
        "gidx": gidx,
    }


def pack_weights(W1, b1, W2, b2, W3, b3, Wl, bl):
    def chunk_rhs(W, kchunks, dtype):
        # [F, 512] -> [128, kchunks, 512]
        F = W.shape[0]
        Wp = np.zeros((kchunks * P, 512), np.float32)
        Wp[:F] = W
        return np.ascontiguousarray(
            Wp.reshape(kchunks, P, 512).transpose(1, 0, 2)
        ).astype(dtype)

    w1cat = np.concatenate([W1[:IN_F], W1[IN_F:]], axis=1)  # [500, 512]
    w2cat = np.concatenate([W2[:HID], W2[HID:]], axis=1)  # [256, 512]
    w3cat = np.concatenate([W3[:HID], W3[HID:]], axis=1)
    wl = np.ascontiguousarray(
        Wl.reshape(6, P, NCLS).transpose(1, 0, 2)
    ).astype(np.float16)  # [128, 6, 7]
    return {
        "w1": chunk_rhs(w1cat, 4, np.float16),
        "w2": chunk_rhs(w2cat, 2, np.float16),
        "w3": chunk_rhs(w3cat, 2, np.float16),
        "wl": wl,
        "b1": np.tile(b1[None, :], (P, 1)).astype(np.float32),
        "b2": np.tile(b2[None, :], (P, 1)).astype(np.float32),
        "b3": np.tile(b3[None, :], (P, 1)).astype(np.float32),
        "bl": np.tile(bl[None, :], (P, 1)).astype(np.float32),
    }


# --------------------------------------------------------------------------
# device kernel
# --------------------------------------------------------------------------

def build(struct):
    b16 = struct["b16"]
    cb = struct["cb"]
    kc = struct["kc"]
    soff = struct["soff"]
    goff = struct["goff"]
    sct = struct["sct"]
    gmax = struct["gmax"]
    cbmax = [int(cb[:, w].max()) for w in range(4)]
    MSG_BUFS = 6

    nc = bacc.Bacc(
        "TRN2", target_bir_lowering=False, debug=False, num_devices=NC,
        num_swdge_queues=4,
    )

    xTc = nc.dram_tensor("xTc", [FPAD, NPC], f16, kind="ExternalInput")
    sd_d = nc.dram_tensor("sd", [P, sct, P], f16, kind="ExternalInput")
    gidx_d = nc.dram_tensor("gidx", [TPC, P, gmax], i16, kind="ExternalInput")
    w1_d = nc.dram_tensor("w1", [P, 4, 512], f16, kind="ExternalInput")
    w2_d = nc.dram_tensor("w2", [P, 2, 512], f16, kind="ExternalInput")
    w3_d = nc.dram_tensor("w3", [P, 2, 512], f16, kind="ExternalInput")
    wl_d = nc.dram_tensor("wl", [P, 6, NCLS], f16, kind="ExternalInput")
    b1_d = nc.dram_tensor("b1", [P, HID], f32, kind="ExternalInput")
    b2_d = nc.dram_tensor("b2", [P, HID], f32, kind="ExternalInput")
    b3_d = nc.dram_tensor("b3", [P, HID], f32, kind="ExternalInput")
    bl_d = nc.dram_tensor("bl", [P, NCLS], f32, kind="ExternalInput")
    out_d = nc.dram_tensor("out", [NPC, NCLS], f32, kind="ExternalOutput")

    with tile.TileContext(nc) as tc:
        with (
            tc.tile_pool(name="dram", bufs=1, space="DRAM") as dram,
            tc.tile_pool(name="const", bufs=1) as cpool,
            tc.tile_pool(name="lx", bufs=6) as lxpool,
            tc.tile_pool(name="stage", bufs=3) as stpool,
            tc.tile_pool(name="msg", bufs=MSG_BUFS) as msgpool,
            tc.tile_pool(name="sbuild", bufs=4) as sbpool,
            tc.tile_pool(name="psa", bufs=2, space="PSUM") as psa,
            tc.tile_pool(name="psagg", bufs=2, space="PSUM") as psagg,
            tc.tile_pool(name="pstr", bufs=2, space="PSUM") as pstr,
            tc.tile_pool(name="pscls", bufs=2, space="PSUM") as pscls,
        ):
            # ---- DRAM intermediates
            p_loc = [
                [dram.tile([HROWS, HID], f16, name=f"p{i}loc{h}")
                 for h in range(2)]
                for i in range(3)
            ]
            p_full = [
                [dram.tile([TAB, HID], f16, addr_space="Shared",
                           name=f"p{i}full{h}") for h in range(2)]
                for i in range(3)
            ]
            r_dram = [dram.tile([NPC, HID], f16, name=f"r{i}d") for i in range(3)]
            x2t_dram = dram.tile([TPC, 2, P, P], f16, name="x2td")

            # ---- constants
            w1_sb = cpool.tile([P, 4, 512], f16)
            nc.sync.dma_start(w1_sb[:], w1_d[:])
            w2_sb = cpool.tile([P, 2, 512], f16)
            nc.sync.dma_start(w2_sb[:], w2_d[:])
            w3_sb = cpool.tile([P, 2, 512], f16)
            nc.sync.dma_start(w3_sb[:], w3_d[:])
            wl_sb = cpool.tile([P, 6, NCLS], f16)
            nc.sync.dma_start(wl_sb[:], wl_d[:])
            b_sb = []
            for name, t in (("b1", b1_d), ("b2", b2_d), ("b3", b3_d)):
                bt = cpool.tile([P, HID], f32, name=name + "sb")
                nc.sync.dma_start(bt[:], t[:])
                b_sb.append(bt)
            bl_sb = cpool.tile([P, NCLS], f32)
            nc.sync.dma_start(bl_sb[:], bl_d[:])

            gidx_sb = cpool.tile([P, TPC, gmax], i16)
            nc.sync.dma_start(
                gidx_sb[:],
                gidx_d[:].rearrange("t p c -> p t c"),
            )

            ident = cpool.tile([P, P], f16)
            make_identity(nc, ident[:])

            x1t_sb = cpool.tile([P, TPC, 2, P], f16)
            lgall = cpool.tile([P, TPC, NCLS], f32)
            sm_all = cpool.tile([P, TPC], f32)
            out_sb = cpool.tile([P, TPC, NCLS], f32)

            # zero-init msg buffers once: untouched (truncated) tail chunks
            # must multiply as 0, never NaN
            for w in range(4):
                for _ in range(MSG_BUFS):
                    mt = msgpool.tile(
                        [P, cbmax[w], HID], f16, name=f"m{w}", tag=f"m{w}"
                    )
                    nc.gpsimd.memset(mt[:], 0.0)

            # ---- phase A (by pairs of tiles): r = x@Wtop + b, p = x@Wbot
            def pair_rows(buf, pr):
                return buf[pr * 2 * P:(pr + 1) * 2 * P, :].rearrange(
                    "(t p) c -> p t c", p=P
                )

            def phase_a_pair(layer, pr):
                half, lpr = pr // (HT // 2), pr % (HT // 2)
                if layer == 0:
                    lxs = []
                    for k in range(4):
                        lx = lxpool.tile([P, 2 * P], f16, name="lx", tag="lx")
                        nc.sync.dma_start(
                            lx[:],
                            xTc[k * P:(k + 1) * P, pr * 2 * P:(pr + 1) * 2 * P],
                        )
                        lxs.append(lx)
                elif layer == 2:
                    x2s = lxpool.tile([P, 4, P], f16, name="x2s", tag="x2s")
                    nc.sync.dma_start(
                        x2s[:],
                        x2t_dram[pr * 2:(pr + 1) * 2].rearrange(
                            "t h p q -> p (t h) q"
                        ),
                    )
                rst = stpool.tile([P, 2, HID], f16, name="rst", tag="rst")
                pst = stpool.tile([P, 2, HID], f16, name="pst", tag="pst")
                for gt in range(2):
                    t = pr * 2 + gt
                    ps = psa.tile([P, 512], f32, name="psA", tag="psA")
                    if layer == 0:
                        for k in range(4):
                            nc.tensor.matmul(
                                out=ps[:], lhsT=lxs[k][:, gt * P:(gt + 1) * P],
                                rhs=w1_sb[:, k, :],
                                start=(k == 0), stop=(k == 3),
                            )
                    elif layer == 1:
                        for k in range(2):
                            nc.tensor.matmul(
                                out=ps[:], lhsT=x1t_sb[:, t, k, :],
                                rhs=w2_sb[:, k, :],
                                start=(k == 0), stop=(k == 1),
                            )
                    else:
                        for k in range(2):
                            nc.tensor.matmul(
                                out=ps[:], lhsT=x2s[:, gt * 2 + k, :],
                                rhs=w3_sb[:, k, :],
                                start=(k == 0), stop=(k == 1),
                            )
                    nc.vector.tensor_tensor(
                        out=rst[:, gt, :], in0=ps[:, :HID], in1=b_sb[layer][:],
                        op=mybir.AluOpType.add,
                    )
                    nc.vector.tensor_copy(pst[:, gt, :], ps[:, HID:])
                nc.sync.dma_start(pair_rows(r_dram[layer], pr), rst[:])
                nc.sync.dma_start(pair_rows(p_loc[layer][half], lpr), pst[:])

            def fire_ag(layer, half):
                nc.gpsimd.collective_compute(
                    "AllGather",
                    mybir.AluOpType.bypass,
                    replica_groups=[list(range(NC))],
                    ins=[p_loc[layer][half].opt()],
                    outs=[p_full[layer][half].opt()],
                )

            # ---- phase B per tile: gather, agg = S^T @ msgs, relu(agg + r)
            def phase_b_tile(layer, tl):
                last = layer == 2
                msgs = []
                for w in range(4):
                    m = msgpool.tile(
                        [P, cbmax[w], HID], f16, name=f"m{w}", tag=f"m{w}"
                    )
                    nb16 = int(b16[tl, w])
                    nc.gpsimd.dma_gather(
                        out_ap=m[:, :int(cb[tl, w]), :],
                        in_ap=p_full[layer][w // 2][
                            (W1BASE if w % 2 else 0):, :
                        ],
                        idxs_ap=gidx_sb[:, tl, goff[tl, w]:goff[tl, w + 1]],
                        num_idxs=nb16,
                        num_idxs_reg=nb16,
                        elem_size=HID,
                        single_packet=(nb16 <= 1024),
                        queue_num=(tl + w) % 4,
                    )
                    msgs.append(m)
                s_t = sbpool.tile([P, int(kc[tl]), P], f16, name="st", tag="st")
                nc.sync.dma_start(
                    s_t[:], sd_d[:, int(soff[tl]):int(soff[tl + 1]), :]
                )
                rst = stpool.tile([P, HID], f16, name="rl", tag="rl")
                nc.sync.dma_start(
                    rst[:],
                    r_dram[layer][tl * P:(tl + 1) * P, :].rearrange(
                        "(o p) c -> p (o c)", p=P
                    ),
                )
                if last:
                    x2c = lxpool.tile([P, 2, P], f16, name="x2c", tag="x2c")
                    nc.sync.dma_start(
                        x2c[:],
                        x2t_dram[tl].rearrange("h p q -> p h q"),
                    )
                agg = psagg.tile([P, HID], f32, name="agg", tag="agg")
                ci = 0
                nchunks = int(kc[tl])
                for w in range(4):
                    for j in range(int(cb[tl, w])):
                        nc.tensor.matmul(
                            out=agg[:],
                            lhsT=s_t[:, ci, :],
                            rhs=msgs[w][:, j, :],
                            start=(ci == 0),
                            stop=(ci == nchunks - 1),
                        )
                        ci += 1
                xsum = stpool.tile([P, HID], f16, name="xsum", tag="xsum")
                nc.vector.tensor_tensor(
                    out=xsum[:], in0=agg[:], in1=rst[:],
                    op=mybir.AluOpType.add,
                )
                xn = stpool.tile([P, HID], f16, name="xn", tag="xn")
                nc.vector.tensor_scalar_max(xn[:], xsum[:], 0.0)
                x3t = []
                for h in range(2):
                    tp = pstr.tile([P, P], f16, name="tp", tag="tp")
                    nc.tensor.transpose(
                        out=tp[:], in_=xn[:, h * P:(h + 1) * P],
                        identity=ident[:],
                    )
                    if layer == 0:
                        nc.vector.tensor_copy(x1t_sb[:, tl, h, :], tp[:])
                    elif layer == 1:
                        xw = stpool.tile([P, P], f16, name=f"x2w{h}",
                                         tag=f"x2w{h}")
                        nc.vector.tensor_copy(xw[:], tp[:])
                        nc.sync.dma_start(
                            x2t_dram[tl, h].rearrange("p q -> p q"), xw[:]
                        )
                    else:
                        xt = stpool.tile([P, P], f16, name="x3t", tag="x3t")
                        nc.vector.tensor_copy(xt[:], tp[:])
                        x3t.append(xt)
                if last:
                    cls = pscls.tile([P, NCLS], f32, name="cls", tag="cls")
                    chunks = [
                        x1t_sb[:, tl, 0, :], x1t_sb[:, tl, 1, :],
                        x2c[:, 0, :], x2c[:, 1, :],
                        x3t[0][:], x3t[1][:],
                    ]
                    for kk in range(6):
                        nc.tensor.matmul(
                            out=cls[:], lhsT=chunks[kk], rhs=wl_sb[:, kk, :],
                            start=(kk == 0), stop=(kk == 5),
                        )
                    nc.vector.tensor_tensor(
                        out=lgall[:, tl, :], in0=cls[:], in1=bl_sb[:],
                        op=mybir.AluOpType.add,
                    )

            # batched shifted-exp over a block of tiles (layer 3)
            def softmax_block(t0, nt):
                mx = stpool.tile([P, nt], f32, name="mx", tag="mx")
                nc.vector.tensor_reduce(
                    out=mx[:], in_=lgall[:, t0:t0 + nt, :],
                    axis=mybir.AxisListType.X, op=mybir.AluOpType.max,
                )
                nc.vector.tensor_tensor(
                    out=lgall[:, t0:t0 + nt, :], in0=lgall[:, t0:t0 + nt, :],
                    in1=mx[:].unsqueeze(2).broadcast_to([P, nt, NCLS]),
                    op=mybir.AluOpType.subtract,
                )
                ex = stpool.tile([P, nt, NCLS], f32, name="ex", tag="ex")
                nc.scalar.activation(
                    ex[:], lgall[:, t0:t0 + nt, :],
                    mybir.ActivationFunctionType.Exp,
                )
                nc.vector.tensor_reduce(
                    out=sm_all[:, t0:t0 + nt], in_=ex[:],
                    axis=mybir.AxisListType.X, op=mybir.AluOpType.add,
                )

            # ---- drive the 3 layers
            for layer in range(3):
                for pr in range(HT):
                    phase_a_pair(layer, pr)
                    if pr == HT // 2 - 1:
                        fire_ag(layer, 0)
                fire_ag(layer, 1)
                for tl in range(TPC):
                    phase_b_tile(layer, tl)
                    if layer == 2 and tl % 4 == 3:
                        softmax_block(tl - 3, 4)

            lsm = cpool.tile([P, TPC], f32)
            nc.scalar.activation(
                lsm[:], sm_all[:], mybir.ActivationFunctionType.Ln
            )
            nc.vector.tensor_tensor(
                out=out_sb[:], in0=lgall[:],
                in1=lsm[:].unsqueeze(2).broadcast_to([P, TPC, NCLS]),
                op=mybir.AluOpType.subtract,
            )
            nc.sync.dma_start(
                out_d[:].rearrange("(t p) j -> p t j", p=P), out_sb[:]
            )

    nc.compile()
    return nc


# --------------------------------------------------------------------------
# entry point
# --------------------------------------------------------------------------

def kernel(x, edge_index, edge_weight, W1, b1, W2, b2, W3, b3, Wl, bl):
    x = np.asarray(x, dtype=np.float32)
    edge_index = np.asarray(edge_index)
    edge_weight = np.asarray(edge_weight, dtype=np.float32)

    pp = prep(x, edge_index, edge_weight)
    wts = pack_weights(
        np.asarray(W1, np.float32), np.asarray(b1, np.float32),
        np.asarray(W2, np.float32), np.asarray(b2, np.float32),
        np.asarray(W3, np.float32), np.asarray(b3, np.float32),
        np.asarray(Wl, np.float32), np.asarray(bl, np.float32),
    )

    key = (pp["sct"], pp["gmax"], tuple(pp["b16"].reshape(-1).tolist()))
    if key not in _compile_cache:
        _compile_cache[key] = build(pp)
    nc = _compile_cache[key]

    in_maps = []
    for c in range(NC):
        in_maps.append({
            "xTc": np.ascontiguousarray(pp["xT"][:, c * NPC:(c + 1) * NPC]),
            "sd": np.ascontiguousarray(pp["sd"][c]),
            "gidx": np.ascontiguousarray(pp["gidx"][c * TPC:(c + 1) * TPC]),
            **wts,
        })

    res = run_bass_kernel_spmd(nc, in_maps, list(range(NC)))
    out_full = np.concatenate([res.results[c]["out"] for c in range(NC)], axis=0)
    return out_full[pp["newpos"][:N]].astype(np.float32)


if __name__ == "__main__":
    import time

    rng = np.random.default_rng(0)
    E = 899756
    ei = rng.integers(0, N, (2, E)).astype(np.int32)
    ew = rng.random(E, dtype=np.float32)
    x = rng.standard_normal((N, IN_F), dtype=np.float32)
    t0 = time.time()
    pp = prep(x, ei, ew)
    print("prep", time.time() - t0, "sct =", pp["sct"], "gmax =", pp["gmax"])


# revision 30
# speedup vs baseline: 1.4235x; 1.1097x over previous
"""3-layer GraphSAGE + classifier + log_softmax on 8 Trainium2 NeuronCores.

Self-contained: host-side sharding/packing + Bass/Tile device kernel.

Strategy
--------
concat([x, agg]) @ W  ==  x @ W_top + Ahat @ (x @ W_bot)   (linearity)
so aggregation happens in the 256-dim projected space.

- Nodes are permuted into 704 tiles of 128 (in-degree balanced), 88 tiles/core.
- Per layer: phase A computes r = x@W_top + b and p = x@W_bot per owned tile.
  p is written fp16 in two halves; each half is AllGathered into its own
  table (45056 rows) as soon as the half is computed, hiding collective
  latency under the remaining phase-A work.
- Phase B: per dst tile, gather p[src] rows for its in-edges via dma_gather
  (int16 indices; 4 overlapping windows, 2 per table). Window 0/2 carry a
  fixed 384 edges per tile (zero padding); windows 1/3 carry the remainder
  with trailing -1 indices that the gather ucode truncates per core.
  Gathers round-robin the 4 SWDGE queues so all 4 Q7 core pairs generate
  descriptors concurrently. The weighted one-hot selection matrices
  S[e, d] = wn_e * (dst_local_e == d) are precomputed on the host and
  DMAed; agg = sum_c S_c.T @ msg_c accumulates on the PE.
  x_next = relu(agg + r).
- x_next is transposed on the PE (2x 128x128) to feed the next layer's
  stationary operand; the classifier (768->7) runs per tile in layer-3
  phase B; the log_softmax runs batched (max/exp/sum per 4-tile block,
  single ln at the end).
"""

import numpy as np

import concourse.bass as bass
import concourse.mybir as mybir
import concourse.tile as tile
from concourse import bacc
from concourse.bass_utils import run_bass_kernel_spmd
from concourse.masks import make_identity

# problem constants
N = 89250
IN_F = 500
HID = 256
NCLS = 7
FPAD = 512  # padded input feature dim

NC = 8  # cores
P = 128
NT = 704  # node tiles
TPC = NT // NC  # 88 tiles per core
NPAD = NT * P  # 90112
NPC = TPC * P  # 11264 nodes per core
HT = TPC // 2  # 44 tiles per half
HROWS = HT * P  # 5632 rows per half per core
TAB = NC * HROWS  # 45056 rows per AllGathered table

WCAP = 32768  # int16 index reach
W1BASE = TAB - WCAP  # 12288: base row of windows 1/3 within their table
SPLIT0 = 384  # fixed edges per tile routed to window 0 (and window 2)
G4 = 4  # tiles per gather group
LA = 2  # lookahead groups for window-0/1 gathers (hide AllGather b)

f32 = mybir.dt.float32
f16 = mybir.dt.float16
i16 = mybir.dt.int16
i32 = mybir.dt.int32

_compile_cache = {}


# --------------------------------------------------------------------------
# host-side prep
# --------------------------------------------------------------------------

def _assign_tiles(in_deg):
    """LPT: assign node ids (0..NPAD) to (tile, slot), balancing in-edges."""
    import heapq

    order = np.argsort(-in_deg, kind="stable")
    heap = [(0, t) for t in range(NT)]
    heapq.heapify(heap)
    counts = np.zeros(NT, np.int32)
    newpos = np.empty(NPAD, np.int64)
    for v in order:
        load, t = heapq.heappop(heap)
        newpos[v] = t * P + counts[t]
        counts[t] += 1
        if counts[t] < P:
            heapq.heappush(heap, (load + int(in_deg[v]), t))
    return newpos


def _ru16(x):
    return (int(x) + 15) // 16 * 16


def prep(x, edge_index, edge_weight):
    src = edge_index[0].astype(np.int64)
    dst = edge_index[1].astype(np.int64)
    ew = edge_weight.astype(np.float32)

    cnt = np.bincount(dst, minlength=N).astype(np.float32)
    wn = ew / np.maximum(cnt[dst], 1.0)

    in_deg = np.zeros(NPAD, np.int64)
    in_deg[:N] = np.bincount(dst, minlength=N)
    newpos = _assign_tiles(in_deg)

    s2 = newpos[src]
    d2 = newpos[dst]
    # table row of each source: owner core c, local row jl; half A = first
    # 44 tiles of the core, half B = rest.  trow = c*HROWS + (jl mod HROWS)
    c_own = s2 // NPC
    jl = s2 % NPC
    is_b = jl >= HROWS
    trow = c_own * HROWS + (jl - HROWS * is_b)

    tile_of = d2 // P
    dl = (d2 % P).astype(np.int64)

    order = np.argsort(tile_of, kind="stable")
    trow_o, isb_o, dl_o, wn_o = trow[order], is_b[order], dl[order], wn[order]
    tile_o = tile_of[order]
    starts = np.searchsorted(tile_o, np.arange(NT + 1))

    # per (tile, window): sorted index lists + (dl, wn) in slot order
    # windows: 0 = A[0:32768), 1 = A[12288:45056), 2/3 same for B
    tw_idx = [[None] * 4 for _ in range(NT)]
    tw_dl = [[None] * 4 for _ in range(NT)]
    tw_wn = [[None] * 4 for _ in range(NT)]
    for t in range(NT):
        lo, hi = starts[t], starts[t + 1]
        tr, ib = trow_o[lo:hi], isb_o[lo:hi]
        dd, ww = dl_o[lo:hi], wn_o[lo:hi]
        for half in range(2):
            sel = np.nonzero(ib == half)[0]
            o = sel[np.argsort(tr[sel], kind="stable")]
            n = len(o)
            assert n >= SPLIT0, f"tile {t} half {half}: only {n} edges"
            assert tr[o[SPLIT0 - 1]] < WCAP, f"tile {t}: w0 split infeasible"
            assert tr[o[SPLIT0]] >= W1BASE, f"tile {t}: w1 split infeasible"
            w0, w1 = 2 * half, 2 * half + 1
            tw_idx[t][w0] = tr[o[:SPLIT0]].astype(np.int16)
            tw_idx[t][w1] = (tr[o[SPLIT0:]] - W1BASE).astype(np.int16)
            tw_dl[t][w0], tw_dl[t][w1] = dd[o[:SPLIT0]], dd[o[SPLIT0:]]
            tw_wn[t][w0], tw_wn[t][w1] = ww[o[:SPLIT0]], ww[o[SPLIT0:]]

    # per tile-slot chunk budgets (uniform across cores for SPMD); tiles are
    # chunk-aligned inside group gathers, so pad each (tile, window) to a
    # multiple of 128 with repeats of the last index (row-buffer-hit reads)
    cb = np.zeros((TPC, 4), np.int64)
    for tl in range(TPC):
        for w in range(4):
            mx = max(len(tw_idx[c * TPC + tl][w]) for c in range(NC))
            cb[tl, w] = (mx + P - 1) // P
    kc = cb.sum(axis=1)  # chunks per tile
    soff = np.zeros(TPC + 1, np.int64)
    soff[1:] = np.cumsum(kc)
    sct = int(soff[-1])

    # group-of-G4 gather packing: per (group, window) one gather whose index
    # list is the concat of the group's tiles (each padded to cb*128)
    ng4 = TPC // G4
    cb4 = np.zeros((ng4, 4), np.int64)  # chunks per (group, window)
    for g in range(ng4):
        cb4[g] = cb[g * G4:(g + 1) * G4].sum(axis=0)
    go4 = np.zeros((ng4, 5), np.int64)  # gidx column offsets (int16 cols)
    for g in range(ng4):
        go4[g, 1:] = np.cumsum(cb4[g] * 8)
    gmax = int(go4[:, 4].max())

    gidx = np.zeros((NC, ng4, P, gmax), np.int16)
    sd = np.zeros((NC, P, sct, P), np.float16)
    for t in range(NT):
        c, tl = t // TPC, t % TPC
        g, ti = tl // G4, tl % G4
        ci0 = 0
        for w in range(4):
            idx = tw_idx[t][w]
            n = len(idx)
            b = int(cb[tl, w]) * P
            arr = np.full(b, idx[-1], np.int16)
            arr[:n] = idx
            wrapped = arr.reshape(-1, 16).T  # [16, b/16]
            coff = int(go4[g, w]) + int(cb[g * G4:tl, w].sum()) * 8
            gidx[c, g, :, coff:coff + b // 16] = np.tile(wrapped, (8, 1))
            sl = np.arange(n)
            sd[c, sl % P, soff[tl] + ci0 + sl // P, tw_dl[t][w]] = tw_wn[t][w]
            ci0 += int(cb[tl, w])

    # transposed, padded, permuted node features
    xT = np.zeros((FPAD, NPAD), np.float16)
    xT[:IN_F, newpos[:N]] = x.T

    return {
        "newpos": newpos,
        "cb": cb,
        "cb4": cb4,
        "go4": go4,
        "kc": kc,
        "soff": soff,
        "sct": sct,
        "gmax": gmax,
        "xT": xT,
        "sd": sd,
        "gidx": gidx,
    }


def pack_weights(W1, b1, W2, b2, W3, b3, Wl, bl):
    def chunk_rhs(W, kchunks, dtype):
        # [F, 512] -> [128, kchunks, 512]
        F = W.shape[0]
        Wp = np.zeros((kchunks * P, 512), np.float32)
        Wp[:F] = W
        return np.ascontiguousarray(
            Wp.reshape(kchunks, P, 512).transpose(1, 0, 2)
        ).astype(dtype)

    w1cat = np.concatenate([W1[:IN_F], W1[IN_F:]], axis=1)  # [500, 512]
    w2cat = np.concatenate([W2[:HID], W2[HID:]], axis=1)  # [256, 512]
    w3cat = np.concatenate([W3[:HID], W3[HID:]], axis=1)
    wl = np.ascontiguousarray(
        Wl.reshape(6, P, NCLS).transpose(1, 0, 2)
    ).astype(np.float16)  # [128, 6, 7]
    return {
        "w1": chunk_rhs(w1cat, 4, np.float16),
        "w2": chunk_rhs(w2cat, 2, np.float16),
        "w3": chunk_rhs(w3cat, 2, np.float16),
        "wl": wl,
        "b1": np.tile(b1[None, :], (P, 1)).astype(np.float32),
        "b2": np.tile(b2[None, :], (P, 1)).astype(np.float32),
        "b3": np.tile(b3[None, :], (P, 1)).astype(np.float32),
        "bl": np.tile(bl[None, :], (P, 1)).astype(np.float32),
    }


# --------------------------------------------------------------------------
# device kernel
# --------------------------------------------------------------------------

def build(struct):
    cb = struct["cb"]
    cb4 = struct["cb4"]
    go4 = struct["go4"]
    kc = struct["kc"]
    soff = struct["soff"]
    sct = struct["sct"]
    gmax = struct["gmax"]
    ng4 = TPC // G4
    cb4max = [int(cb4[:, w].max()) for w in range(4)]

    nc = bacc.Bacc(
        "TRN2", target_bir_lowering=False, debug=False, num_devices=NC,
        num_swdge_queues=4,
    )

    xTc = nc.dram_tensor("xTc", [FPAD, NPC], f16, kind="ExternalInput")
    sd_d = nc.dram_tensor("sd", [P, sct, P], f16, kind="ExternalInput")
    gidx_d = nc.dram_tensor("gidx", [ng4, P, gmax], i16, kind="ExternalInput")
    w1_d = nc.dram_tensor("w1", [P, 4, 512], f16, kind="ExternalInput")
    w2_d = nc.dram_tensor("w2", [P, 2, 512], f16, kind="ExternalInput")
    w3_d = nc.dram_tensor("w3", [P, 2, 512], f16, kind="ExternalInput")
    wl_d = nc.dram_tensor("wl", [P, 6, NCLS], f16, kind="ExternalInput")
    b1_d = nc.dram_tensor("b1", [P, HID], f32, kind="ExternalInput")
    b2_d = nc.dram_tensor("b2", [P, HID], f32, kind="ExternalInput")
    b3_d = nc.dram_tensor("b3", [P, HID], f32, kind="ExternalInput")
    bl_d = nc.dram_tensor("bl", [P, NCLS], f32, kind="ExternalInput")
    out_d = nc.dram_tensor("out", [NPC, NCLS], f32, kind="ExternalOutput")

    with tile.TileContext(nc) as tc:
        with (
            tc.tile_pool(name="dram", bufs=1, space="DRAM") as dram,
            tc.tile_pool(name="const", bufs=1) as cpool,
            tc.tile_pool(name="lx", bufs=4) as lxpool,
            tc.tile_pool(name="stage", bufs=3) as stpool,
            tc.tile_pool(name="msga", bufs=LA + 1) as msgapool,
            tc.tile_pool(name="msgb", bufs=2) as msgbpool,
            tc.tile_pool(name="sbuild", bufs=2) as sbpool,
            tc.tile_pool(name="psa", bufs=2, space="PSUM") as psa,
            tc.tile_pool(name="psagg", bufs=2, space="PSUM") as psagg,
            tc.tile_pool(name="pstr", bufs=2, space="PSUM") as pstr,
            tc.tile_pool(name="pscls", bufs=2, space="PSUM") as pscls,
        ):
            # ---- DRAM intermediates
            p_loc = [
                [dram.tile([HROWS, HID], f16, name=f"p{i}loc{h}")
                 for h in range(2)]
                for i in range(3)
            ]
            p_full = [
                [dram.tile([TAB, HID], f16, addr_space="Shared",
                           name=f"p{i}full{h}") for h in range(2)]
                for i in range(3)
            ]
            r_dram = [dram.tile([NPC, HID], f16, name=f"r{i}d") for i in range(3)]
            x2t_dram = dram.tile([TPC, 2, P, P], f16, name="x2td")

            # ---- constants
            w1_sb = cpool.tile([P, 4, 512], f16)
            nc.sync.dma_start(w1_sb[:], w1_d[:])
            w2_sb = cpool.tile([P, 2, 512], f16)
            nc.sync.dma_start(w2_sb[:], w2_d[:])
            w3_sb = cpool.tile([P, 2, 512], f16)
            nc.sync.dma_start(w3_sb[:], w3_d[:])
            wl_sb = cpool.tile([P, 6, NCLS], f16)
            nc.sync.dma_start(wl_sb[:], wl_d[:])
            b_sb = []
            for name, t in (("b1", b1_d), ("b2", b2_d), ("b3", b3_d)):
                bt = cpool.tile([P, HID], f32, name=name + "sb")
                nc.sync.dma_start(bt[:], t[:])
                b_sb.append(bt)
            bl_sb = cpool.tile([P, NCLS], f32)
            nc.sync.dma_start(bl_sb[:], bl_d[:])

            gidx_sb = cpool.tile([P, ng4, gmax], i16)
            nc.sync.dma_start(
                gidx_sb[:],
                gidx_d[:].rearrange("g p c -> p g c"),
            )

            ident = cpool.tile([P, P], f16)
            make_identity(nc, ident[:])

            x1t_sb = cpool.tile([P, TPC, 2, P], f16)
            lgall = cpool.tile([P, TPC, NCLS], f32)
            sm_all = cpool.tile([P, TPC], f32)
            out_sb = cpool.tile([P, TPC, NCLS], f32)

            # ---- phase A (by pairs of tiles): r = x@Wtop + b, p = x@Wbot
            def pair_rows(buf, pr):
                return buf[pr * 2 * P:(pr + 1) * 2 * P, :].rearrange(
                    "(t p) c -> p t c", p=P
                )

            def phase_a_pair(layer, pr):
                half, lpr = pr // (HT // 2), pr % (HT // 2)
                if layer == 0:
                    lxs = []
                    for k in range(4):
                        lx = lxpool.tile([P, 2 * P], f16, name="lx", tag="lx")
                        nc.sync.dma_start(
                            lx[:],
                            xTc[k * P:(k + 1) * P, pr * 2 * P:(pr + 1) * 2 * P],
                        )
                        lxs.append(lx)
                elif layer == 2:
                    x2s = lxpool.tile([P, 4, P], f16, name="x2s", tag="x2s")
                    nc.sync.dma_start(
                        x2s[:],
                        x2t_dram[pr * 2:(pr + 1) * 2].rearrange(
                            "t h p q -> p (t h) q"
                        ),
                    )
                rst = stpool.tile([P, 2, HID], f16, name="rst", tag="rst")
                pst = stpool.tile([P, 2, HID], f16, name="pst", tag="pst")
                for gt in range(2):
                    t = pr * 2 + gt
                    ps = psa.tile([P, 512], f32, name="psA", tag="psA")
                    if layer == 0:
                        for k in range(4):
                            nc.tensor.matmul(
                                out=ps[:], lhsT=lxs[k][:, gt * P:(gt + 1) * P],
                                rhs=w1_sb[:, k, :],
                                start=(k == 0), stop=(k == 3),
                            )
                    elif layer == 1:
                        for k in range(2):
                            nc.tensor.matmul(
                                out=ps[:], lhsT=x1t_sb[:, t, k, :],
                                rhs=w2_sb[:, k, :],
                                start=(k == 0), stop=(k == 1),
                            )
                    else:
                        for k in range(2):
                            nc.tensor.matmul(
                                out=ps[:], lhsT=x2s[:, gt * 2 + k, :],
                                rhs=w3_sb[:, k, :],
                                start=(k == 0), stop=(k == 1),
                            )
                    nc.vector.tensor_tensor(
                        out=rst[:, gt, :], in0=ps[:, :HID], in1=b_sb[layer][:],
                        op=mybir.AluOpType.add,
                    )
                    nc.vector.tensor_copy(pst[:, gt, :], ps[:, HID:])
                nc.sync.dma_start(pair_rows(r_dram[layer], pr), rst[:])
                nc.sync.dma_start(pair_rows(p_loc[layer][half], lpr), pst[:])

            def fire_ag(layer, half):
                nc.gpsimd.collective_compute(
                    "AllGather",
                    mybir.AluOpType.bypass,
                    replica_groups=[list(range(NC))],
                    ins=[p_loc[layer][half].opt()],
                    outs=[p_full[layer][half].opt()],
                )

            # ---- phase B by groups of G4 tiles
            def issue_gathers(layer, g, ws):
                ms = {}
                for w in ws:
                    pool = msgapool if w < 2 else msgbpool
                    m = pool.tile(
                        [P, cb4max[w], HID], f16, name=f"m{w}", tag=f"m{w}"
                    )
                    cols = int(cb4[g, w])
                    ni = cols * P
                    nc.gpsimd.dma_gather(
                        out_ap=m[:, :cols, :],
                        in_ap=p_full[layer][w // 2][
                            (W1BASE if w % 2 else 0):, :
                        ],
                        idxs_ap=gidx_sb[:, g, int(go4[g, w]):int(go4[g, w + 1])],
                        num_idxs=ni,
                        num_idxs_reg=ni,
                        elem_size=HID,
                        single_packet=(ni <= 1024),
                        queue_num=w,
                    )
                    ms[w] = m
                return ms

            def phase_b_group(layer, g, msgs):
                last = layer == 2
                t0 = g * G4
                kcg = int(soff[t0 + G4] - soff[t0])
                s_t = sbpool.tile([P, kcg, P], f16, name="st", tag="st")
                nc.sync.dma_start(
                    s_t[:], sd_d[:, int(soff[t0]):int(soff[t0 + G4]), :]
                )
                rst = stpool.tile([P, G4, HID], f16, name="rl", tag="rl")
                nc.sync.dma_start(
                    rst[:],
                    r_dram[layer][t0 * P:(t0 + G4) * P, :].rearrange(
                        "(t p) c -> p t c", p=P
                    ),
                )
                if last:
                    x2c = lxpool.tile([P, 2 * G4, P], f16, name="x2c", tag="x2c")
                    nc.sync.dma_start(
                        x2c[:],
                        x2t_dram[t0:t0 + G4].rearrange("t h p q -> p (t h) q"),
                    )
                for ti in range(G4):
                    tl = t0 + ti
                    agg = psagg.tile([P, HID], f32, name="agg", tag="agg")
                    ci = int(soff[tl] - soff[t0])
                    nchunks = int(kc[tl])
                    done = 0
                    for w in range(4):
                        cbo = int(cb[t0:tl, w].sum())
                        for j in range(int(cb[tl, w])):
                            nc.tensor.matmul(
                                out=agg[:],
                                lhsT=s_t[:, ci, :],
                                rhs=msgs[w][:, cbo + j, :],
                                start=(done == 0),
                                stop=(done == nchunks - 1),
                            )
                            ci += 1
                            done += 1
                    xsum = stpool.tile([P, HID], f16, name="xsum", tag="xsum")
                    nc.vector.tensor_tensor(
                        out=xsum[:], in0=agg[:], in1=rst[:, ti, :],
                        op=mybir.AluOpType.add,
                    )
                    xn = stpool.tile([P, HID], f16, name="xn", tag="xn")
                    nc.scalar.activation(
                        xn[:], xsum[:], mybir.ActivationFunctionType.Relu
                    )
                    x3t = []
                    for h in range(2):
                        tp = pstr.tile([P, P], f16, name="tp", tag="tp")
                        nc.tensor.transpose(
                            out=tp[:], in_=xn[:, h * P:(h + 1) * P],
                            identity=ident[:],
                        )
                        if layer == 0:
                            nc.vector.tensor_copy(x1t_sb[:, tl, h, :], tp[:])
                        elif layer == 1:
                            xw = stpool.tile([P, P], f16, name=f"x2w{h}",
                                             tag=f"x2w{h}")
                            nc.vector.tensor_copy(xw[:], tp[:])
                            nc.sync.dma_start(
                                x2t_dram[tl, h].rearrange("p q -> p q"), xw[:]
                            )
                        else:
                            xt = stpool.tile([P, P], f16, name="x3t", tag="x3t")
                            nc.vector.tensor_copy(xt[:], tp[:])
                            x3t.append(xt)
                    if last:
                        cls = pscls.tile([P, NCLS], f32, name="cls", tag="cls")
                        chunks = [
                            x1t_sb[:, tl, 0, :], x1t_sb[:, tl, 1, :],
                            x2c[:, ti * 2, :], x2c[:, ti * 2 + 1, :],
                            x3t[0][:], x3t[1][:],
                        ]
                        for kk in range(6):
                            nc.tensor.matmul(
                                out=cls[:], lhsT=chunks[kk], rhs=wl_sb[:, kk, :],
                                start=(kk == 0), stop=(kk == 5),
                            )
                        nc.vector.tensor_tensor(
                            out=lgall[:, tl, :], in0=cls[:], in1=bl_sb[:],
                            op=mybir.AluOpType.add,
                        )

            # batched shifted-exp over a block of tiles (layer 3)
            def softmax_block(t0, nt):
                mx = stpool.tile([P, nt], f32, name="mx", tag="mx")
                nc.vector.tensor_reduce(
                    out=mx[:], in_=lgall[:, t0:t0 + nt, :],
                    axis=mybir.AxisListType.X, op=mybir.AluOpType.max,
                )
                nc.vector.tensor_tensor(
                    out=lgall[:, t0:t0 + nt, :], in0=lgall[:, t0:t0 + nt, :],
                    in1=mx[:].unsqueeze(2).broadcast_to([P, nt, NCLS]),
                    op=mybir.AluOpType.subtract,
                )
                ex = stpool.tile([P, nt, NCLS], f32, name="ex", tag="ex")
                nc.scalar.activation(
                    ex[:], lgall[:, t0:t0 + nt, :],
                    mybir.ActivationFunctionType.Exp,
                )
                nc.vector.tensor_reduce(
                    out=sm_all[:, t0:t0 + nt], in_=ex[:],
                    axis=mybir.AxisListType.X, op=mybir.AluOpType.add,
                )

            # ---- drive the 3 layers
            for layer in range(3):
                for pr in range(HT):
                    phase_a_pair(layer, pr)
                    if pr == HT // 2 - 1:
                        fire_ag(layer, 0)
                fire_ag(layer, 1)
                # lookahead window-0/1 gathers overlap AllGather b's flight
                amsgs = [issue_gathers(layer, g, (0, 1)) for g in range(LA)]
                for g in range(ng4):
                    msgs = issue_gathers(layer, g, (2, 3))
                    if g + LA < ng4:
                        amsgs.append(issue_gathers(layer, g + LA, (0, 1)))
                    msgs.update(amsgs[g])
                    phase_b_group(layer, g, msgs)
                    if layer == 2:
                        softmax_block(g * G4, G4)

            lsm = cpool.tile([P, TPC], f32)
            nc.scalar.activation(
                lsm[:], sm_all[:], mybir.ActivationFunctionType.Ln
            )
            nc.vector.tensor_tensor(
                out=out_sb[:], in0=lgall[:],
                in1=lsm[:].unsqueeze(2).broadcast_to([P, TPC, NCLS]),
                op=mybir.AluOpType.subtract,
            )
            nc.sync.dma_start(
                out_d[:].rearrange("(t p) j -> p t j", p=P), out_sb[:]
            )

    nc.compile()
    return nc


# --------------------------------------------------------------------------
# entry point
# --------------------------------------------------------------------------

def kernel(x, edge_index, edge_weight, W1, b1, W2, b2, W3, b3, Wl, bl):
    x = np.asarray(x, dtype=np.float32)
    edge_index = np.asarray(edge_index)
    edge_weight = np.asarray(edge_weight, dtype=np.float32)

    pp = prep(x, edge_index, edge_weight)
    wts = pack_weights(
        np.asarray(W1, np.float32), np.asarray(b1, np.float32),
        np.asarray(W2, np.float32), np.asarray(b2, np.float32),
        np.asarray(W3, np.float32), np.asarray(b3, np.float32),
        np.asarray(Wl, np.float32), np.asarray(bl, np.float32),
    )

    key = (pp["sct"], pp["gmax"], tuple(pp["cb"].reshape(-1).tolist()))
    if key not in _compile_cache:
        _compile_cache[key] = build(pp)
    nc = _compile_cache[key]

    in_maps = []
    for c in range(NC):
        in_maps.append({
            "xTc": np.ascontiguousarray(pp["xT"][:, c * NPC:(c + 1) * NPC]),
            "sd": np.ascontiguousarray(pp["sd"][c]),
            "gidx": np.ascontiguousarray(pp["gidx"][c]),
            **wts,
        })

    res = run_bass_kernel_spmd(nc, in_maps, list(range(NC)))
    out_full = np.concatenate([res.results[c]["out"] for c in range(NC)], axis=0)
    return out_full[pp["newpos"][:N]].astype(np.float32)


if __name__ == "__main__":
    import time

    rng = np.random.default_rng(0)
    E = 899756
    ei = rng.integers(0, N, (2, E)).astype(np.int32)
    ew = rng.random(E, dtype=np.float32)
    x = rng.standard_normal((N, IN_F), dtype=np.float32)
    t0 = time.time()
    pp = prep(x, ei, ew)
    print("prep", time.time() - t0, "sct =", pp["sct"], "gmax =", pp["gmax"])
